# revision 1
# baseline (speedup 1.0000x reference)
"""CapsuleLayer (dynamic routing) Trainium2 kernel — 8 NeuronCores.

Strategy: shard over input capsules I (2048 -> 256/core). W-load drops to
8 MB/core (bf16: 4 MB). Routing softmax/logit state is per-(b, i, j) and thus
core-local; the three routing reductions s_r = sum_i c*u_hat are computed as
per-core partials on the tensor engine and AllReduce'd (128 KB) across cores.

Per-core pipeline:
  P1  u_hat einsum: block-diag(x) [128,128] @ W-octet [128,512] matmuls,
      PSUM -> bf16 SBUF, rearranged to [i-partition, (b, j*k)] via DRAM bounce.
  P2  round 0: uniform-c weighted sums on PE -> diag-extract -> AllReduce ->
      squash(v0) computed redundantly on every core.
  P3  rounds 1,2: b-logit update on vector engine (TT mul + segment reduce),
      softmax (ACT exp + reciprocal), c-weighted sums on PE (col-tiled 4x),
      AllReduce, squash. Round 2's v is the output.
"""
import numpy as np
import ml_dtypes
from contextlib import ExitStack

import concourse.bass as bass
import concourse.mybir as mybir
import concourse.tile as tile
from concourse import bacc
from concourse import bass_utils

B, I, D, J, Kd = 64, 2048, 16, 32, 16
NCORES = 8
IC = I // NCORES      # 256 input capsules per core
NCH = 2               # i-chunks of 128 per core
NOCT = 16             # octets of 8 i per chunk
NSUB = 4              # sub-batches of b
BS = B // NSUB        # 16
JK = J * Kd           # 512
EPS = 1e-7
USE_FOLD = False
BF16 = mybir.dt.bfloat16
F32 = mybir.dt.float32
AX = mybir.AxisListType
OP = mybir.AluOpType
ACTF = mybir.ActivationFunctionType


def _host_prep(inputs, W, core):
    """Per-core DMA-ready layouts (bf16)."""
    Wc = W[core * IC:(core + 1) * IC]  # [256, 32, 16, 16] = [i, j, d, k]
    # wl[ch, oct, (il, d), (j, k)]
    wl = Wc.reshape(NCH, NOCT, 8, J, D, Kd).transpose(0, 1, 2, 4, 3, 5) \
           .reshape(NCH, NOCT, 128, JK)
    wl = np.ascontiguousarray(wl).astype(ml_dtypes.bfloat16)

    xc = inputs[:, core * IC:(core + 1) * IC, :]  # [64, 256, 16] = [b, i, d]
    # xr[ch, oct, sub, il, d, bs]
    xr = xc.reshape(NSUB, BS, NCH, NOCT, 8, D).transpose(2, 3, 0, 4, 5, 1)
    xbd = np.zeros((NCH, NOCT, NSUB, 128, 128), np.float32)
    for il in range(8):
        # rows (il,d) = il*16+d ; cols m = bs*8+il
        xbd[:, :, :, il * 16:(il + 1) * 16, il::8] = xr[:, :, :, il]
    return wl, xbd.astype(ml_dtypes.bfloat16)


def _host_bd16():
    # ones-blockdiag lhsT for the fused s0 reduction: bd16[(bs*8+il), bs'] =
    # (1/J) * (bs == bs')  -> psum[bs', jk] = (1/J) sum_il tmp[(bs,il), jk]
    bd = np.zeros((128, BS), np.float32)
    for bs in range(BS):
        bd[bs * 8:(bs + 1) * 8, bs] = 1.0 / J
    return bd.astype(ml_dtypes.bfloat16)


def _squash_emit(nc, pool, tiny, src_ap, out_dtype, nb=B):
    """Emit squash on s tile [nb, 512] fp32 view [nb, 32, 16]; returns v tile."""
    sq = pool.tile([nb, JK], F32, tag="sq")
    nc.vector.tensor_mul(sq[:], src_ap, src_ap)
    nn = tiny.tile([nb, J], F32, tag="nn")
    nc.vector.tensor_reduce(nn[:], sq[:].rearrange("b (j k) -> b j k", k=Kd),
                            axis=AX.X, op=OP.add)
    t1 = tiny.tile([nb, J], F32, tag="t1")
    nc.vector.tensor_scalar_add(t1[:], nn[:], 1.0)
    t2 = tiny.tile([nb, J], F32, tag="t2")
    nc.vector.tensor_scalar_add(t2[:], nn[:], EPS)
    st = tiny.tile([nb, J], F32, tag="st")
    nc.scalar.sqrt(st[:], t2[:])
    den = tiny.tile([nb, J], F32, tag="den")
    nc.vector.tensor_mul(den[:], t1[:], st[:])
    rden = tiny.tile([nb, J], F32, tag="rden")
    nc.vector.reciprocal(rden[:], den[:])
    sc = tiny.tile([nb, J], F32, tag="sc")
    nc.vector.tensor_mul(sc[:], nn[:], rden[:])
    v = pool.tile([nb, JK], out_dtype, tag="vout")
    nc.vector.tensor_mul(
        v[:].rearrange("b (j k) -> b j k", k=Kd),
        src_ap.rearrange("b (j k) -> b j k", k=Kd),
        sc[:, :, None].broadcast_to([nb, J, Kd]))
    return v


def build_program(collectives=True):
    nc = bacc.Bacc("TRN2", target_bir_lowering=False, debug=False,
                   num_devices=NCORES if collectives else 1)
    wl_d = nc.dram_tensor("wl", [NCH, NOCT, 128, JK], BF16, kind="ExternalInput")
    xbd_d = nc.dram_tensor("xbd", [NCH, NOCT, NSUB, 128, 128], BF16,
                           kind="ExternalInput")
    bd16_d = nc.dram_tensor("bd16", [128, BS], BF16, kind="ExternalInput")
    out_d = nc.dram_tensor("out", [B // NCORES, J, Kd], F32, kind="ExternalOutput")

    with tile.TileContext(nc) as tc, ExitStack() as ctx:
        dram = ctx.enter_context(tc.tile_pool(name="dram", bufs=1, space="DRAM"))
        wpool = ctx.enter_context(tc.tile_pool(name="wp", bufs=3))
        xpool = ctx.enter_context(tc.tile_pool(name="xp", bufs=4))
        epsum = ctx.enter_context(tc.tile_pool(name="ep", bufs=2, space="PSUM"))
        s0psum = ctx.enter_context(tc.tile_pool(name="s0p", bufs=1, space="PSUM"))
        spsum = ctx.enter_context(tc.tile_pool(name="sp", bufs=2, space="PSUM"))
        tmpp = ctx.enter_context(tc.tile_pool(name="tm", bufs=2))
        drp = ctx.enter_context(tc.tile_pool(name="drp", bufs=2))
        uhp = ctx.enter_context(tc.tile_pool(name="uh", bufs=1))
        rp = ctx.enter_context(tc.tile_pool(name="rp", bufs=3))
        smp = ctx.enter_context(tc.tile_pool(name="smp", bufs=5))
        tiny = ctx.enter_context(tc.tile_pool(name="ty", bufs=4))
        vp = ctx.enter_context(tc.tile_pool(name="vp", bufs=1))

        ub = dram.tile([NCH, NOCT, NSUB, 128, JK], BF16)
        sstage = dram.tile([B, J, JK], F32)
        arin = dram.tile([B, J, Kd], F32)
        arout = dram.tile([B, J, Kd], F32)
        vd = dram.tile([B, JK], BF16)

        u_hat = [uhp.tile([128, B, JK], BF16, tag=f"uh{c}", name=f"u_hat{c}")
                 for c in range(NCH)]
        bb = [uhp.tile([128, B, J], F32, tag=f"bb{c}", name=f"bb{c}")
              for c in range(NCH)]

        bd16 = rp.tile([128, BS], BF16, tag="bd16")
        nc.sync.dma_start(bd16[:], bd16_d[:])

        # ---------------- P1: einsum + fused s0 partials ----------------
        s0ps = [s0psum.tile([BS, JK], F32, tag=f"s0p{s}", name=f"s0ps{s}")
                for s in range(NSUB)]
        for ch in range(NCH):
            for oc in range(NOCT):
                wt = wpool.tile([128, JK], BF16)
                nc.sync.dma_start(wt[:], wl_d[ch, oc])
                xt4 = xpool.tile([128, NSUB * 128], BF16)
                xb = xbd_d[ch, oc]  # [NSUB, 128, 128]
                xsrc = bass.AP(tensor=xb.tensor, offset=xb.offset,
                               ap=[[128, 128], [128 * 128, NSUB], [1, 128]])
                nc.sync.dma_start(xt4[:], xsrc)
                tm4 = tmpp.tile([128, NSUB * JK], BF16)
                for sub in range(NSUB):
                    pe = epsum.tile([128, JK], F32)
                    nc.tensor.matmul(pe[:], xt4[:, sub * 128:(sub + 1) * 128],
                                     wt[:], start=True, stop=True)
                    tm = tm4[:, sub * JK:(sub + 1) * JK]
                    if sub % 2 == 0:
                        nc.scalar.copy(tm, pe[:])
                    else:
                        nc.vector.tensor_copy(tm, pe[:])
                    # fused s0 partial: psum[bs,jk] += (1/J) sum_il tm[(bs,il),jk]
                    nc.tensor.matmul(s0ps[sub][:], bd16[:], tm,
                                     start=(ch == 0 and oc == 0),
                                     stop=(ch == NCH - 1 and oc == NOCT - 1))
                if USE_FOLD:
                    # direct sbuf->sbuf partition fold: for each bs, move the
                    # 8 partitions (bs*8+il) into u_hat[oc*8+il] at b=sub*16+bs
                    tview = tm4[:].rearrange("p (s f) -> p s f", f=JK)
                    uview = u_hat[ch][oc * 8:(oc + 1) * 8, :, :]
                    for bs in range(BS):
                        nc.sync.dma_start(
                            uview[:, bs::BS, :],
                            tview[bs * 8:(bs + 1) * 8, :, :])
                else:
                    base = ub[:]
                    blk = (ch * NOCT + oc) * NSUB * 128 * JK
                    udst = bass.AP(tensor=base.tensor, offset=base.offset + blk,
                                   ap=[[JK, 128], [128 * JK, NSUB], [1, JK]])
                    nc.sync.dma_start(udst, tm4[:])
                    # readback -> u_hat[ch][oc*8+il, (sub,bs), :]
                    usrc = bass.AP(tensor=base.tensor, offset=base.offset + blk,
                                   ap=[[JK, 8], [128 * JK, NSUB], [8 * JK, BS], [1, JK]])
                    nc.sync.dma_start(u_hat[ch][oc * 8:(oc + 1) * 8, :, :], usrc)
        # drain s0: sstage[b, j', jk] needs [32, 512] per b; s0 psum rows are
        # identical across j' only for the diag trick -- instead write the
        # 512-vector straight to the diag target: arin[b, j, k] = s0[b, j*16+k].
        for sub in range(NSUB):
            s0sb = drp.tile([BS, JK], F32, tag="s0sb", name=f"s0sb{sub}")
            nc.scalar.copy(s0sb[:], s0ps[sub][:])
            nc.sync.dma_start(
                arin[:].rearrange("b j k -> b (j k)")[sub * BS:(sub + 1) * BS, :],
                s0sb[:])

        rsout = dram.tile([B // NCORES, J, Kd], F32)

        def all_reduce(last=False):
            if collectives:
                if last:
                    nc.gpsimd.collective_compute(
                        "ReduceScatter", OP.add,
                        replica_groups=[list(range(NCORES))],
                        ins=[arin.opt()], outs=[rsout.opt()])
                else:
                    nc.gpsimd.collective_compute(
                        "AllReduce", OP.add,
                        replica_groups=[list(range(NCORES))],
                        ins=[arin.opt()], outs=[arout.opt()])
            else:
                if last:
                    nc.sync.dma_start(rsout[:],
                                      arin[:][0:B // NCORES])
                else:
                    nc.sync.dma_start(arout[:], arin[:])

        # ---------------- rounds ----------------
        for r in range(3):
            all_reduce(last=(r == 2))
            if r < 2:
                sv = vp.tile([B, JK], F32, tag="sv", name=f"sv{r}")
                nc.sync.dma_start(sv[:], arout[:].rearrange("b j k -> b (j k)"))
                v = _squash_emit(nc, vp, tiny, sv[:], BF16)
                nc.sync.dma_start(vd[:], v[:])
            else:
                svs = vp.tile([B // NCORES, JK], F32, tag="svs", name="svs")
                nc.sync.dma_start(svs[:], rsout[:].rearrange("b j k -> b (j k)"))
                v = _squash_emit(nc, vp, tiny, svs[:], F32, nb=B // NCORES)
                nc.sync.dma_start(out_d[:].rearrange("b j k -> b (j k)"), v[:])
                break

            # next round: bb update + softmax + weighted sums, batched by
            # groups of 4 consecutive b
            for g in range(B // 4):
                ps = spsum.tile([128, JK], F32, tag="spsum", name=f"sp{r}_{g}")
                vb4 = rp.tile([128, 4 * JK], BF16, tag="vb4", name=f"vb{r}_{g}")
                vsrc = vd[:]
                vap = bass.AP(tensor=vsrc.tensor, offset=vsrc.offset + g * 4 * JK,
                              ap=[[0, 128], [JK, 4], [1, JK]])
                nc.sync.dma_start(vb4[:], vap)
                for ch in range(NCH):
                    prod = rp.tile([128, 4 * JK], BF16, tag="prod",
                                   name=f"pr{r}_{g}_{ch}")
                    eng = nc.vector if ch == 0 else nc.gpsimd
                    eng.tensor_mul(prod[:],
                                   u_hat[ch][:, g * 4:(g + 1) * 4, :]
                                   .rearrange("p b f -> p (b f)"),
                                   vb4[:])
                    bbs = bb[ch][:, g * 4:(g + 1) * 4, :]  # [128, 4, 32]
                    if r == 0:
                        with nc.allow_low_precision("bb accum in fp32 out"):
                            nc.vector.tensor_reduce(
                                bbs,
                                prod[:].rearrange("p (bj k) -> p bj k", k=Kd),
                                axis=AX.X, op=OP.add)
                    else:
                        binc = smp.tile([128, 4 * J], F32, tag="binc",
                                       name=f"bi{r}_{g}_{ch}")
                        nc.vector.tensor_reduce(
                            binc[:].rearrange("p (bj o) -> p bj o", o=1)
                            if False else binc[:].rearrange(
                                "p (bj) -> p bj", bj=4 * J),
                            prod[:].rearrange("p (bj k) -> p bj k", k=Kd),
                            axis=AX.X, op=OP.add)
                        nc.gpsimd.tensor_add(
                            bbs.rearrange("p b j -> p (b j)"),
                            bbs.rearrange("p b j -> p (b j)"), binc[:])
                    e4 = smp.tile([128, 4 * J], BF16, tag="e4",
                                 name=f"e{r}_{g}_{ch}")
                    nc.scalar.activation(e4[:],
                                         bbs.rearrange("p b j -> p (b j)"),
                                         ACTF.Exp)
                    z4 = tiny.tile([128, 4], F32, tag="z4", name=f"z{r}_{g}_{ch}")
                    nc.vector.tensor_reduce(
                        z4[:], e4[:].rearrange("p (b j) -> p b j", j=J),
                        axis=AX.X, op=OP.add)
                    rz4 = tiny.tile([128, 4], F32, tag="rz4",
                                    name=f"rz{r}_{g}_{ch}")
                    nc.vector.reciprocal(rz4[:], z4[:])
                    c4 = smp.tile([128, 4 * J], BF16, tag="c4",
                                 name=f"c{r}_{g}_{ch}")
                    nc.vector.tensor_mul(
                        c4[:].rearrange("p (b j) -> p b j", j=J),
                        e4[:].rearrange("p (b j) -> p b j", j=J),
                        rz4[:, :, None].broadcast_to([128, 4, J]))
                    for bq in range(4):
                        b = g * 4 + bq
                        nc.tensor.matmul(ps[bq * 32:(bq + 1) * 32, :],
                                         c4[:, bq * J:(bq + 1) * J],
                                         u_hat[ch][:, b, :],
                                         start=(ch == 0), stop=(ch == 1),
                                         tile_position=(0, bq * 32),
                                         skip_group_check=True)
                sdr = drp.tile([128, JK], F32, tag="sdr", name=f"sd{r}_{g}")
                nc.scalar.copy(sdr[:], ps[:])
                nc.sync.dma_start(sstage[:][g * 4:(g + 1) * 4], sdr[:])
                sbase = sstage[:]
                diag = bass.AP(tensor=sbase.tensor,
                               offset=sbase.offset + g * 4 * J * JK,
                               ap=[[J * JK, 4], [JK + Kd, J], [1, Kd]])
                nc.sync.dma_start(arin[:][g * 4:(g + 1) * 4], diag)

    nc.compile()
    return nc


_NC_CACHE = None


_RUN_CACHE = None


def kernel(inputs, W, routings=3):
    """Full inputs in, full [B, J, K] output out. Shards over I across the
    8 NeuronCores internally; first call compiles and caches the executable."""
    global _NC_CACHE, _RUN_CACHE
    import jax
    from jax.sharding import NamedSharding, PartitionSpec
    inputs = np.asarray(inputs, dtype=np.float32)
    W = np.asarray(W, dtype=np.float32)
    if _NC_CACHE is None:
        _NC_CACHE = build_program()
    nc = _NC_CACHE
    if _RUN_CACHE is None:
        _RUN_CACHE = _build_sharded(nc)
    fn, mesh, in_names, out_names, out_avals, zero_outs = _RUN_CACHE
    per_core = []
    for core in range(NCORES):
        wl, xbd = _host_prep(inputs, W, core)
        per_core.append({"wl": wl, "xbd": xbd, "bd16": _host_bd16()})
    sh = NamedSharding(mesh, PartitionSpec("core"))
    concat_in = [jax.device_put(
        np.concatenate([per_core[c][n] for c in range(NCORES)], axis=0), sh)
        for n in in_names]
    zeros = [jax.device_put(
        np.zeros((NCORES * z.shape[0], *z.shape[1:]), z.dtype), sh)
        for z in zero_outs]
    out = fn(*concat_in, *zeros)
    jax.block_until_ready(out)
    oidx = out_names.index("out")
    return np.asarray(out[oidx]).reshape(B, J, Kd)


# ---------------- timing harness (test-only) ----------------
def _build_sharded(nc):
    """Replicate bass2jax.run_bass_via_pjrt's jit construction, returning
    (fn, in_names, out_names, out_avals, n_params)."""
    import jax
    from jax.sharding import Mesh, PartitionSpec
    from jax.experimental.shard_map import shard_map
    from concourse import bass2jax as b2j
    from concourse.bass2jax import _bass_exec_p, install_neuronx_cc_hook, partition_id_tensor
    install_neuronx_cc_hook()
    partition_name = nc.partition_id_tensor.name if nc.partition_id_tensor else None
    in_names, out_names, out_avals, zero_outs = [], [], [], []
    for alloc in nc.m.functions[0].allocations:
        if not isinstance(alloc, mybir.MemoryLocationSet):
            continue
        name = alloc.memorylocations[0].name
        if alloc.kind == "ExternalInput":
            if name != partition_name:
                in_names.append(name)
        elif alloc.kind == "ExternalOutput":
            out_names.append(name)
            shape = tuple(alloc.tensor_shape)
            dtype = mybir.dt.np(alloc.dtype)
            out_avals.append(jax.core.ShapedArray(shape, dtype))
            zero_outs.append(np.zeros(shape, dtype))
    n_params = len(in_names)
    n_outs = len(out_avals)
    all_in = list(in_names) + list(out_names)
    if partition_name is not None:
        all_in.append(partition_name)
    donate = tuple(range(n_params, n_params + n_outs))

    def _body(*args):
        operands = list(args)
        if partition_name is not None:
            operands.append(partition_id_tensor())
        return tuple(_bass_exec_p.bind(
            *operands, out_avals=tuple(out_avals), in_names=tuple(all_in),
            out_names=tuple(out_names), lowering_input_output_aliases=(),
            sim_require_finite=True, sim_require_nnan=True, nc=nc))

    devices = jax.devices()[:NCORES]
    mesh = Mesh(np.array(devices), ("core",))
    in_specs = (PartitionSpec("core"),) * (n_params + n_outs)
    out_specs = (PartitionSpec("core"),) * n_outs
    fn = jax.jit(shard_map(_body, mesh=mesh, in_specs=in_specs,
                           out_specs=out_specs, check_rep=False),
                 donate_argnums=donate, keep_unused=True)
    return fn, mesh, in_names[:n_params], out_names, out_avals, zero_outs


def timed_run(inputs, W, iters=20):
    """Returns (best_ns, times_ns list, output)."""
    import time, jax
    from jax.sharding import NamedSharding, PartitionSpec
    nc = build_program() if _NC_CACHE is None else _NC_CACHE
    fn, mesh, in_names, out_names, out_avals, zero_outs = _build_sharded(nc)
    per_core = []
    for core in range(NCORES):
        wl, xbd = _host_prep(inputs, W, core)
        per_core.append({"wl": wl, "xbd": xbd, "bd16": _host_bd16()})
    sh = NamedSharding(mesh, PartitionSpec("core"))
    concat_in = [jax.device_put(
        np.concatenate([per_core[c][n] for c in range(NCORES)], axis=0), sh)
        for n in in_names]
    def make_zeros():
        return [jax.device_put(
            np.zeros((NCORES * z.shape[0], *z.shape[1:]), z.dtype), sh)
            for z in zero_outs]
    zsets = [make_zeros() for _ in range(iters + 3)]
    out = None
    times = []
    for it in range(iters + 3):
        t0 = time.perf_counter_ns()
        res = fn(*concat_in, *zsets[it])
        jax.block_until_ready(res)
        dt = time.perf_counter_ns() - t0
        if it >= 3:
            times.append(dt)
        out = res
    out_np = np.asarray(out[0]).reshape(B, J, Kd)
    return min(times), times, out_np



# revision 36
# speedup vs baseline: 1.3518x; 1.3518x over previous
"""CapsuleLayer (dynamic routing) Trainium2 kernel — 8 NeuronCores.

Strategy: shard over input capsules I (2048 -> 256/core); W load 4 MB/core
(bf16). Routing state is per-(b, i, j) and core-local; the three routing
reductions s_r are per-core PE partials AllReduce'd (bf16, 64 KB).

Per-core pipeline:
  P1  u_hat einsum: block-diag(x) [128,128] @ W-octet [128,512] matmuls;
      psum -> bf16 sbuf -> DRAM bounce (big-descriptor DMAs) -> resident
      u_hat [i-part, (b, jk)] bf16 (128 KiB/partition).
      s0 = (1/J) sum_i u_hat computed directly as xT @ W matmuls (no
      u_hat dependence), AllReduce, squash -> v0 (redundant per core).
  P2  rounds 1,2: logits = reduce_k(u_hat * w_bcast) on DVE in 4x bf16
      mode; softmax staged j-major so the 1/z multiply stays in 4x mode;
      c-weighted sums on PE (col-tiled 4x per 4-batch); diag extraction
      via one staging DMA + a 7ns/descriptor DRAM->DRAM gather;
      AllReduce (round 1) / ReduceScatter (round 2) -> squash.
"""
import numpy as np
import ml_dtypes
from contextlib import ExitStack

import concourse.bass as bass
import concourse.mybir as mybir
import concourse.tile as tile
from concourse import bacc
from concourse import bass_utils

B, I, D, J, Kd = 64, 2048, 16, 32, 16
NCORES = 8
IC = I // NCORES      # 256 input capsules per core
NCH = 2               # i-chunks of 128 per core
NOCT = 16             # octets of 8 i per chunk
NSUB = 4              # sub-batches of b
BS = B // NSUB        # 16
JK = J * Kd           # 512
NG8 = 8               # groups of 8 batch elements
EPS = 1e-7
BF16 = mybir.dt.bfloat16
F32 = mybir.dt.float32
AX = mybir.AxisListType
OP = mybir.AluOpType
ACTF = mybir.ActivationFunctionType


def _host_prep(inputs, W, core):
    """Per-core DMA-ready layouts (bf16)."""
    bf = ml_dtypes.bfloat16
    Wc = W[core * IC:(core + 1) * IC]  # [256, 32, 16, 16] = [i, j, d, k]
    # wl2[(il,d), ch, oc, (j,k)]
    wl2 = Wc.reshape(NCH, NOCT, 8, J, D, Kd).transpose(2, 4, 0, 1, 3, 5) \
            .reshape(128, NCH, NOCT, JK)
    xc = inputs[:, core * IC:(core + 1) * IC, :]  # [64, 256, 16] = [b, i, d]
    xr = xc.reshape(B, NCH, NOCT, 8, D)           # [b, ch, oc, il, d]
    # xbd2[(il,d), ch, oc, sub, bs*8+il] = x[sub*16+bs, i(ch,oc,il), d]
    xbd2 = np.zeros((128, NCH, NOCT, NSUB, 128), np.float32)
    for il in range(8):
        blk = xr[:, :, :, il, :].reshape(NSUB, BS, NCH, NOCT, D) \
                                .transpose(4, 2, 3, 0, 1)
        xbd2[il * 16:(il + 1) * 16, :, :, :, il::8] = blk
    # xT2[(il,d), ch, oc, b]
    xT2 = np.ascontiguousarray(xr.transpose(3, 4, 1, 2, 0)).reshape(
        128, NCH, NOCT, B)
    return {"wl2": np.ascontiguousarray(wl2).astype(bf),
            "xbd2": xbd2.astype(bf),
            "xT2": np.ascontiguousarray(xT2).astype(bf)}


def _squash_emit(nc, pool, tiny, src_ap, out_dtype, nb=B, tag=""):
    """squash on s [nb, 512] viewed [nb, 32, 16]; returns v tile [nb, 512]."""
    sq = pool.tile([nb, JK], BF16, tag="sq")
    nc.vector.tensor_mul(sq[:], src_ap, src_ap)
    nn = tiny.tile([nb, J], F32, tag="nn")
    nc.vector.tensor_reduce(nn[:], sq[:].rearrange("b (j k) -> b j k", k=Kd),
                            axis=AX.X, op=OP.add)
    t1 = tiny.tile([nb, J], F32, tag="t1")
    nc.vector.tensor_scalar_add(t1[:], nn[:], 1.0)
    t2 = tiny.tile([nb, J], F32, tag="t2")
    nc.vector.tensor_scalar_add(t2[:], nn[:], EPS)
    st = tiny.tile([nb, J], F32, tag="st")
    nc.scalar.sqrt(st[:], t2[:])
    den = tiny.tile([nb, J], F32, tag="den")
    nc.vector.tensor_mul(den[:], t1[:], st[:])
    rden = tiny.tile([nb, J], F32, tag="rden")
    nc.vector.reciprocal(rden[:], den[:])
    sc = tiny.tile([nb, J], F32, tag="sc")
    nc.vector.tensor_mul(sc[:], nn[:], rden[:])
    v = pool.tile([nb, JK], out_dtype, tag="vout" + ("f" if out_dtype == F32 else ""))
    nc.vector.tensor_mul(
        v[:].rearrange("b (j k) -> b j k", k=Kd),
        src_ap.rearrange("b (j k) -> b j k", k=Kd),
        sc[:, :, None].broadcast_to([nb, J, Kd]))
    return v


def build_program(collectives=True):
    nc = bacc.Bacc("TRN2", target_bir_lowering=False, debug=False,
                   num_devices=NCORES if collectives else 1)
    wl2_d = nc.dram_tensor("wl2", [128, NCH, NOCT, JK], BF16,
                           kind="ExternalInput")
    xbd2_d = nc.dram_tensor("xbd2", [128, NCH, NOCT, NSUB, 128], BF16,
                            kind="ExternalInput")
    xT2_d = nc.dram_tensor("xT2", [128, NCH, NOCT, B], BF16,
                           kind="ExternalInput")
    out_d = nc.dram_tensor("out", [B // NCORES, J, Kd], F32,
                           kind="ExternalOutput")

    with tile.TileContext(nc) as tc, ExitStack() as ctx:
        dram = ctx.enter_context(tc.tile_pool(name="dram", bufs=1, space="DRAM"))
        uhp = ctx.enter_context(tc.tile_pool(name="uh", bufs=1))
        tiny = ctx.enter_context(tc.tile_pool(name="ty", bufs=2))
        vp = ctx.enter_context(tc.tile_pool(name="vp", bufs=1))
        p1ctx = ExitStack()
        wpool = p1ctx.enter_context(tc.tile_pool(name="wp", bufs=1))
        xpool = p1ctx.enter_context(tc.tile_pool(name="xp", bufs=1))
        xtp = p1ctx.enter_context(tc.tile_pool(name="xtp", bufs=1))
        epsum = p1ctx.enter_context(tc.tile_pool(name="ep", bufs=3, space="PSUM"))
        s0psum = p1ctx.enter_context(tc.tile_pool(name="s0p", bufs=1, space="PSUM"))
        tmpp = p1ctx.enter_context(tc.tile_pool(name="tm", bufs=3))

        # bounce layout: addr = ((ch*16+bs)*8+il)*32768 + oc*2048 + sub*512 + e
        # write partitions (bs,il) stride 32768; read partitions (il,oc)
        # stride 2048 -> both sides are 3-dim APs.
        ub = dram.tile([NCH, BS, 8, NOCT, NSUB, JK], BF16)
        sstage = [dram.tile([2 * NG8, 128, JK], BF16, name=f"sst{r}")
                  for r in (1, 2)]
        arin = [dram.tile([B, JK], BF16, name=f"arin{r}") for r in range(3)]
        arout = [dram.tile([B, JK], BF16, name=f"arout{r}") for r in range(2)]
        rsout = dram.tile([B // NCORES, JK], BF16)
        wd = [dram.tile([B, JK], BF16, name=f"wd{r}") for r in (1, 2)]

        # resident tiles
        wres = wpool.tile([128, NCH, NOCT, JK], BF16, tag="wres")
        nc.sync.dma_start(wres[:, 0], wl2_d[:, 0])
        nc.sync.dma_start(wres[:, 1], wl2_d[:, 1])
        xT = xtp.tile([128, NCH, NOCT, B], BF16, tag="xT")
        nc.sync.dma_start(xT[:], xT2_d[:])
        u_hat = [uhp.tile([128, NSUB * BS, JK], BF16, tag=f"uh{c}",
                          name=f"u_hat{c}") for c in range(NCH)]

        # ---------------- P1: einsum + bounce; s0 between chunk passes ----
        # NOTE: GPSIMD/Pool cannot access PSUM on real HW -- drains must
        # stay on DVE/Act.
        drain_ch = [[nc.vector, nc.scalar], [nc.scalar, nc.vector]]
        s0ps = s0psum.tile([B, JK], F32, tag="s0ps")

        def einsum_ch(ch):
            for ocg in range(4):
                xbd = xpool.tile([128, 4, NSUB, 128], BF16, tag="xbd")
                nc.sync.dma_start(xbd[:], xbd2_d[:, ch, ocg * 4:(ocg + 1) * 4])
                for oo in range(4):
                    oc = ocg * 4 + oo
                    wt = wres[:, ch, oc, :]
                    tm4 = tmpp.tile([128, NSUB, JK], BF16, tag="tm4")
                    for half in range(2):
                        pe = epsum.tile([128, 2 * JK], F32, tag="pe")
                        for s2 in range(2):
                            sub = half * 2 + s2
                            nc.tensor.matmul(pe[:, s2 * JK:(s2 + 1) * JK],
                                             xbd[:, oo, sub, :], wt,
                                             start=True, stop=True,
                                             skip_group_check=True)
                        eng = drain_ch[ch][half]
                        dst = tm4[:, half * 2:(half + 1) * 2, :] \
                            .rearrange("p s f -> p (s f)")
                        if eng is nc.scalar:
                            eng.copy(dst, pe[:])
                        else:
                            eng.tensor_copy(dst, pe[:])
                    # bounce write: partitions (bs,il) at stride 32768
                    base = ub[:]
                    wdst = bass.AP(
                        tensor=base.tensor,
                        offset=(base.offset + ch * BS * 8 * 32768
                                + oc * NSUB * JK),
                        ap=[[NOCT * NSUB * JK, 128], [JK, NSUB], [1, JK]])
                    nc.sync.dma_start(wdst, tm4[:])

        def read_ch(ch):
            # bounce read: partitions (il,oc) at stride 2048, per sub;
            # issued from the Act queue so SP can keep streaming writes
            base = ub[:]
            for sub in range(NSUB):
                off = base.offset + ch * BS * 8 * 32768 + sub * JK
                src = bass.AP(tensor=base.tensor, offset=off,
                              ap=[[NSUB * JK, 128], [8 * 32768, BS], [1, JK]])
                nc.scalar.dma_start(
                    u_hat[ch][:, sub * BS:(sub + 1) * BS, :], src)

        einsum_ch(0)
        # s0 on PE after the ch0 einsum stream
        for ch in range(NCH):
            for oc in range(NOCT):
                nc.tensor.matmul(s0ps[:], xT[:, ch, oc, :], wres[:, ch, oc, :],
                                 start=(ch == 0 and oc == 0),
                                 stop=(ch == NCH - 1 and oc == NOCT - 1))
        read_ch(0)
        s0sb = vp.tile([B, JK], BF16, tag="s0sb")
        nc.scalar.mul(s0sb[:], s0ps[:], 1.0 / J)
        nc.scalar.dma_start(arin[0][:], s0sb[:])
        einsum_ch(1)

        def all_reduce(idx, last=False):
            if collectives:
                if last:
                    nc.gpsimd.collective_compute(
                        "ReduceScatter", OP.add,
                        replica_groups=[list(range(NCORES))],
                        ins=[arin[idx].opt()], outs=[rsout.opt()])
                else:
                    nc.gpsimd.collective_compute(
                        "AllReduce", OP.add,
                        replica_groups=[list(range(NCORES))],
                        ins=[arin[idx].opt()], outs=[arout[idx].opt()])
            else:
                if last:
                    nc.gpsimd.dma_start(rsout[:], arin[idx][:][0:B // NCORES])
                else:
                    nc.gpsimd.dma_start(arout[idx][:], arin[idx][:])

        all_reduce(0)

        # ---------------- rounds 1, 2 ----------------
        wacc = vp.tile([B, JK], F32, tag="wacc")

        def vchain(r):
            # squash previous round's s -> v_{r-1}; build w_r staging
            sv = vp.tile([B, JK], BF16, tag="sv")
            nc.scalar.dma_start(sv[:], arout[r - 1][:])
            v = _squash_emit(nc, vp, tiny, sv[:], BF16, tag=f"r{r}")
            if r == 1:
                nc.vector.tensor_copy(wacc[:], v[:])
                nc.scalar.dma_start(wd[0][:], v[:])
            else:
                nc.gpsimd.tensor_add(wacc[:], wacc[:], v[:])
                wb = vp.tile([B, JK], BF16, tag="wb")
                nc.vector.tensor_copy(wb[:], wacc[:])
                nc.scalar.dma_start(wd[1][:], wb[:])

        def round_body(r):
            wsrc = wd[r - 1][:]
            sdr = sdp.tile([128, 2 * NG8, JK], BF16, tag="sdr")
            for g in range(NG8):
                # broadcast w[8b, 512] to all partitions
                wexp = wxp.tile([128, 8 * JK], BF16, tag="wexp")
                nc.gpsimd.dma_start(
                    wexp[:],
                    bass.AP(tensor=wsrc.tensor,
                            offset=wsrc.offset + g * 8 * JK,
                            ap=[[0, 128], [1, 8 * JK]]))
                pss = [spsum.tile([128, JK], F32, tag=f"ps{bq}",
                                  name=f"ps{r}_{g}_{bq}") for bq in range(2)]
                for ch in range(NCH):
                    uslc = u_hat[ch][:, g * 8:(g + 1) * 8, :] \
                        .rearrange("p b f -> p (b f)")
                    veng = nc.vector
                    prod = prp.tile([128, 8 * JK], BF16, tag="prod")
                    veng.tensor_mul(prod[:], uslc, wexp[:])
                    # k-reduction as a 2x-mode TT add tree (TensorReduce has
                    # no DVE perf modes -> 4x slower than this)
                    pv = prod[:].rearrange("p (bj k) -> p bj k", k=Kd)
                    t1 = smp.tile([128, 256, 8], BF16, tag="tr1")
                    veng.tensor_add(t1[:], pv[:, :, 0:8], pv[:, :, 8:16])
                    t2 = smp.tile([128, 256, 4], BF16, tag="tr2")
                    veng.tensor_add(t2[:], t1[:, :, 0:4], t1[:, :, 4:8])
                    t3 = smp.tile([128, 256, 2], BF16, tag="tr3")
                    veng.tensor_add(t3[:], t2[:, :, 0:2], t2[:, :, 2:4])
                    lg = smp.tile([128, 8, J], BF16, tag="lg")
                    veng.tensor_add(
                        lg[:].rearrange("p b j -> p (b j)"),
                        t3[:, :, 0], t3[:, :, 1])
                    # softmax, staged j-major so 1/z multiply stays 4x
                    est = smp.tile([128, J, 8], BF16, tag="est")
                    nc.scalar.activation(est[:].rearrange("p j b -> p b j"),
                                         lg[:], ACTF.Exp)
                    z = tiny.tile([128, 8], BF16, tag="z")
                    with nc.allow_low_precision("bf16 softmax z"):
                        nc.vector.tensor_reduce(
                            z[:], est[:].rearrange("p j b -> p b j"),
                            axis=AX.X, op=OP.add)
                    rz = tiny.tile([128, 8], BF16, tag="rz")
                    with nc.allow_low_precision("bf16 softmax 1/z"):
                        nc.vector.reciprocal(rz[:], z[:])
                    cst = smp.tile([128, J, 8], BF16, tag="cst")
                    nc.vector.tensor_mul(
                        cst[:], est[:],
                        rz[:, None, :].broadcast_to([128, J, 8]))
                    # c-weighted sums: 2 bq-groups x 4 col-tiled matmuls
                    for bq in range(2):
                        ps = pss[bq]
                        for b4 in range(4):
                            b = g * 8 + bq * 4 + b4
                            nc.tensor.matmul(
                                ps[b4 * 32:(b4 + 1) * 32, :],
                                cst[:, :, bq * 4 + b4],
                                u_hat[ch][:, b, :],
                                start=(ch == 0), stop=(ch == 1),
                                tile_position=(0, b4 * 32),
                                skip_group_check=True)
                        if ch == 1:
                            g2 = g * 2 + bq
                            nc.scalar.copy(sdr[:, g2, :], ps[:])
                            # pipelined staging + diag gather for this slice
                            nc.sync.dma_start(sstage[r - 1][g2],
                                              sdr[:, g2, :])
                            sb = sstage[r - 1][:]
                            diag = bass.AP(
                                tensor=sb.tensor,
                                offset=sb.offset + g2 * 128 * JK,
                                ap=[[32 * JK, 4], [JK + Kd, J], [1, Kd]])
                            ab = arin[r][:]
                            adst = bass.AP(
                                tensor=ab.tensor,
                                offset=ab.offset + (g * 8 + bq * 4) * JK,
                                ap=[[JK, 4], [Kd, J], [1, Kd]])
                            nc.sync.dma_start(adst, diag)

        vchain(1)
        read_ch(1)
        p1ctx.close()
        spsum = ctx.enter_context(tc.tile_pool(name="sp", bufs=3, space="PSUM"))
        wxp = ctx.enter_context(tc.tile_pool(name="wx", bufs=2))
        prp = ctx.enter_context(tc.tile_pool(name="pr", bufs=2))
        smp = ctx.enter_context(tc.tile_pool(name="smp", bufs=2))
        sdp = ctx.enter_context(tc.tile_pool(name="sdp", bufs=1))
        round_body(1)
        all_reduce(1)
        vchain(2)
        round_body(2)
        all_reduce(2, last=True)

        # final squash on this core's 8 batch rows
        svf = vp.tile([B // NCORES, JK], BF16, tag="svf")
        nc.scalar.dma_start(svf[:], rsout[:])
        vout = _squash_emit(nc, vp, tiny, svf[:], F32, nb=B // NCORES,
                            tag="fin")
        nc.scalar.dma_start(out_d[:].rearrange("b j k -> b (j k)"), vout[:])

    nc.compile()
    return nc


_NC_CACHE = None
_RUN_CACHE = None


def kernel(inputs, W, routings=3):
    """Full inputs in, full [B, J, K] output out. Shards over I across the
    8 NeuronCores internally; first call compiles and caches the executable."""
    global _NC_CACHE, _RUN_CACHE
    import jax
    from jax.sharding import NamedSharding, PartitionSpec
    inputs = np.asarray(inputs, dtype=np.float32)
    W = np.asarray(W, dtype=np.float32)
    if _NC_CACHE is None:
        _NC_CACHE = build_program()
    nc = _NC_CACHE
    if _RUN_CACHE is None:
        _RUN_CACHE = _build_sharded(nc)
    fn, mesh, in_names, out_names, out_avals, zero_outs = _RUN_CACHE
    per_core = [_host_prep(inputs, W, core) for core in range(NCORES)]
    sh = NamedSharding(mesh, PartitionSpec("core"))
    concat_in = [jax.device_put(
        np.concatenate([per_core[c][n] for c in range(NCORES)], axis=0), sh)
        for n in in_names]
    zeros = [jax.device_put(
        np.zeros((NCORES * z.shape[0], *z.shape[1:]), z.dtype), sh)
        for z in zero_outs]
    out = fn(*concat_in, *zeros)
    jax.block_until_ready(out)
    oidx = out_names.index("out")
    return np.asarray(out[oidx]).reshape(B, J, Kd)


# ---------------- timing harness (test-only) ----------------
def _build_sharded(nc):
    """Replicate bass2jax.run_bass_via_pjrt's jit construction, returning
    (fn, mesh, in_names, out_names, out_avals, zero_outs)."""
    import jax
    from jax.sharding import Mesh, PartitionSpec
    from jax.experimental.shard_map import shard_map
    from concourse.bass2jax import (_bass_exec_p, install_neuronx_cc_hook,
                                    partition_id_tensor)
    install_neuronx_cc_hook()
    partition_name = (nc.partition_id_tensor.name
                      if nc.partition_id_tensor else None)
    in_names, out_names, out_avals, zero_outs = [], [], [], []
    for alloc in nc.m.functions[0].allocations:
        if not isinstance(alloc, mybir.MemoryLocationSet):
            continue
        name = alloc.memorylocations[0].name
        if alloc.kind == "ExternalInput":
            if name != partition_name:
                in_names.append(name)
        elif alloc.kind == "ExternalOutput":
            out_names.append(name)
            shape = tuple(alloc.tensor_shape)
            dtype = mybir.dt.np(alloc.dtype)
            out_avals.append(jax.core.ShapedArray(shape, dtype))
            zero_outs.append(np.zeros(shape, dtype))
    n_params = len(in_names)
    n_outs = len(out_avals)
    all_in = list(in_names) + list(out_names)
    if partition_name is not None:
        all_in.append(partition_name)
    donate = tuple(range(n_params, n_params + n_outs))

    def _body(*args):
        operands = list(args)
        if partition_name is not None:
            operands.append(partition_id_tensor())
        return tuple(_bass_exec_p.bind(
            *operands, out_avals=tuple(out_avals), in_names=tuple(all_in),
            out_names=tuple(out_names), lowering_input_output_aliases=(),
            sim_require_finite=True, sim_require_nnan=True, nc=nc))

    devices = jax.devices()[:NCORES]
    mesh = Mesh(np.array(devices), ("core",))
    in_specs = (PartitionSpec("core"),) * (n_params + n_outs)
    out_specs = (PartitionSpec("core"),) * n_outs
    fn = jax.jit(shard_map(_body, mesh=mesh, in_specs=in_specs,
                           out_specs=out_specs, check_rep=False),
                 donate_argnums=donate, keep_unused=True)
    return fn, mesh, in_names[:n_params], out_names, out_avals, zero_outs


def timed_run(inputs, W, iters=20):
    """Returns (best_ns, times_ns list, output)."""
    import time, jax
    from jax.sharding import NamedSharding, PartitionSpec
    global _NC_CACHE, _RUN_CACHE
    if _NC_CACHE is None:
        _NC_CACHE = build_program()
    nc = _NC_CACHE
    if _RUN_CACHE is None:
        _RUN_CACHE = _build_sharded(nc)
    fn, mesh, in_names, out_names, out_avals, zero_outs = _RUN_CACHE
    per_core = [_host_prep(inputs, W, core) for core in range(NCORES)]
    sh = NamedSharding(mesh, PartitionSpec("core"))
    concat_in = [jax.device_put(
        np.concatenate([per_core[c][n] for c in range(NCORES)], axis=0), sh)
        for n in in_names]
    def make_zeros():
        return [jax.device_put(
            np.zeros((NCORES * z.shape[0], *z.shape[1:]), z.dtype), sh)
            for z in zero_outs]
    zsets = [make_zeros() for _ in range(iters + 3)]
    out = None
    times = []
    for it in range(iters + 3):
        t0 = time.perf_counter_ns()
        res = fn(*concat_in, *zsets[it])
        jax.block_until_ready(res)
        dt = time.perf_counter_ns() - t0
        if it >= 3:
            times.append(dt)
        out = res
    oidx = out_names.index("out")
    out_np = np.asarray(out[oidx]).reshape(B, J, Kd)
    return min(times), times, out_np


# revision 59
# speedup vs baseline: 1.5812x; 1.1697x over previous
"""CapsuleLayer (dynamic routing) Trainium2 kernel — 8 NeuronCores.

Strategy: shard over input capsules I (2048 -> 256/core); W load 4 MB/core
(bf16). Routing state is per-(b, i, j) and core-local; the three routing
reductions s_r are per-core PE partials, AllReduce'd in bf16 (64 KB).

Per-core pipeline (ordered to keep the in-order engine/DMA queues clear):
  P1  u_hat einsum: block-diag(x) [128,128] @ W-octet [128,512] matmuls,
      two N=512 col-tiled matmuls per 2-bank psum tile; psum drained as
      [128,1024] bf16 copies (DVE+Act only -- GPSIMD cannot touch PSUM);
      DRAM bounce with 3-dim APs on both sides (write partitions (bs,il)
      stride 32768, read partitions (il,oc) stride 2048) -> resident
      u_hat[ch] [i-part, (b, jk)] bf16, 128 KiB/partition.
      s0 = (1/J) sum_i u_hat as 32 direct xT @ W matmuls emitted between
      the two einsum chunk passes; its AllReduce + squash(v0) + w-staging
      complete while the bounce streams.
  P2  rounds 1,2 (8 groups of 8 batch x 2 i-chunks):
      logits = reduce_k(u_hat * w_bcast): TT multiply in 2x bf16 mode +
      a 4-level TT add tree (TensorReduce has no DVE perf modes and would
      be 4x slower); softmax staged j-major so the 1/z multiply stays in
      packed mode; c-weighted sums as 4 col-tiled PE matmuls per 4-batch
      sharing one psum tile across both i-chunks; per-slice staging DMA +
      7ns/descriptor diagonal gather DRAM->DRAM, pipelined into the round;
      AllReduce (r1) / ReduceScatter (r2) -> squash -> [8, 512] out.
      DMA issue is spread across queues: SP = bounce + staging + wexp-free
      slots, Act = bounce reads (ch0) + v-chain, Pool/SWDGE = wexp
      broadcasts, so no queue head-of-line-blocks another phase.
"""
import numpy as np
import ml_dtypes
from contextlib import ExitStack

import concourse.bass as bass
import concourse.mybir as mybir
import concourse.tile as tile
from concourse import bacc
from concourse import bass_utils

B, I, D, J, Kd = 64, 2048, 16, 32, 16
NCORES = 8
IC = I // NCORES      # 256 input capsules per core
NCH = 2               # i-chunks of 128 per core
NOCT = 16             # octets of 8 i per chunk
NSUB = 4              # sub-batches of b
BS = B // NSUB        # 16
JK = J * Kd           # 512
NG8 = 8               # groups of 8 batch elements
EPS = 1e-7
BF16 = mybir.dt.bfloat16
F32 = mybir.dt.float32
AX = mybir.AxisListType
OP = mybir.AluOpType
ACTF = mybir.ActivationFunctionType


def _host_prep(inputs, W, core):
    """Per-core DMA-ready layouts (bf16)."""
    bf = ml_dtypes.bfloat16
    Wc = W[core * IC:(core + 1) * IC]  # [256, 32, 16, 16] = [i, j, d, k]
    # wl2[(il,d), ch, oc, (j,k)]
    wl2 = Wc.reshape(NCH, NOCT, 8, J, D, Kd).transpose(2, 4, 0, 1, 3, 5) \
            .reshape(128, NCH, NOCT, JK)
    xc = inputs[:, core * IC:(core + 1) * IC, :]  # [64, 256, 16] = [b, i, d]
    xr = xc.reshape(B, NCH, NOCT, 8, D)           # [b, ch, oc, il, d]
    # xbd2[(il,d), ch, oc, sub, bs*8+il] = x[sub*16+bs, i(ch,oc,il), d]
    xbd2 = np.zeros((128, NCH, NOCT, NSUB, 128), np.float32)
    for il in range(8):
        blk = xr[:, :, :, il, :].reshape(NSUB, BS, NCH, NOCT, D) \
                                .transpose(4, 2, 3, 0, 1)
        xbd2[il * 16:(il + 1) * 16, :, :, :, il::8] = blk
    # xT2[(il,d), ch, oc, b]
    xT2 = np.ascontiguousarray(xr.transpose(3, 4, 1, 2, 0)).reshape(
        128, NCH, NOCT, B)
    return {"wl2": np.ascontiguousarray(wl2).astype(bf),
            "xbd2": xbd2.astype(bf),
            "xT2": np.ascontiguousarray(xT2).astype(bf)}


def _squash_emit(nc, pool, tiny, src_ap, out_dtype, nb=B, tag=""):
    """squash on s [nb, 512] viewed [nb, 32, 16]; returns v tile [nb, 512]."""
    sq = pool.tile([nb, JK], BF16, tag="sq")
    nc.vector.tensor_mul(sq[:], src_ap, src_ap)
    nn = tiny.tile([nb, J], F32, tag="nn")
    nc.vector.tensor_reduce(nn[:], sq[:].rearrange("b (j k) -> b j k", k=Kd),
                            axis=AX.X, op=OP.add)
    t1 = tiny.tile([nb, J], F32, tag="t1")
    nc.vector.tensor_scalar_add(t1[:], nn[:], 1.0)
    t2 = tiny.tile([nb, J], F32, tag="t2")
    nc.vector.tensor_scalar_add(t2[:], nn[:], EPS)
    st = tiny.tile([nb, J], F32, tag="st")
    nc.scalar.sqrt(st[:], t2[:])
    den = tiny.tile([nb, J], F32, tag="den")
    nc.vector.tensor_mul(den[:], t1[:], st[:])
    rden = tiny.tile([nb, J], F32, tag="rden")
    nc.vector.reciprocal(rden[:], den[:])
    sc = tiny.tile([nb, J], F32, tag="sc")
    nc.vector.tensor_mul(sc[:], nn[:], rden[:])
    v = pool.tile([nb, JK], out_dtype, tag="vout" + ("f" if out_dtype == F32 else ""))
    nc.vector.tensor_mul(
        v[:].rearrange("b (j k) -> b j k", k=Kd),
        src_ap.rearrange("b (j k) -> b j k", k=Kd),
        sc[:, :, None].broadcast_to([nb, J, Kd]))
    return v


def build_program(collectives=True):
    nc = bacc.Bacc("TRN2", target_bir_lowering=False, debug=False,
                   num_devices=NCORES if collectives else 1)
    wl2_d = nc.dram_tensor("wl2", [128, NCH, NOCT, JK], BF16,
                           kind="ExternalInput")
    xbd2_d = nc.dram_tensor("xbd2", [128, NCH, NOCT, NSUB, 128], BF16,
                            kind="ExternalInput")
    xT2_d = nc.dram_tensor("xT2", [128, NCH, NOCT, B], BF16,
                           kind="ExternalInput")
    out_d = nc.dram_tensor("out", [B // NCORES, J, Kd], F32,
                           kind="ExternalOutput")

    with tile.TileContext(nc) as tc, ExitStack() as ctx:
        dram = ctx.enter_context(tc.tile_pool(name="dram", bufs=1, space="DRAM"))
        uhp = ctx.enter_context(tc.tile_pool(name="uh", bufs=1))
        tiny = ctx.enter_context(tc.tile_pool(name="ty", bufs=3))
        vp = ctx.enter_context(tc.tile_pool(name="vp", bufs=1))
        p1ctx = ExitStack()
        wpool = p1ctx.enter_context(tc.tile_pool(name="wp", bufs=1))
        xpool = p1ctx.enter_context(tc.tile_pool(name="xp", bufs=2))
        xtp = p1ctx.enter_context(tc.tile_pool(name="xtp", bufs=1))
        epsum = p1ctx.enter_context(tc.tile_pool(name="ep", bufs=3, space="PSUM"))
        s0psum = p1ctx.enter_context(tc.tile_pool(name="s0p", bufs=1, space="PSUM"))
        tmpp = p1ctx.enter_context(tc.tile_pool(name="tm", bufs=4))

        # bounce layout: addr = ((ch*16+bs)*8+il)*32768 + oc*2048 + sub*512 + e
        # write partitions (bs,il) stride 32768; read partitions (il,oc)
        # stride 2048 -> both sides are 3-dim APs.
        ub = dram.tile([NCH, BS, 8, NOCT, NSUB, JK], BF16)
        sstage = [dram.tile([2 * NG8, 128, JK], BF16, name=f"sst{r}")
                  for r in (1, 2)]
        arin = [dram.tile([B, JK], BF16, name=f"arin{r}") for r in range(3)]
        arout = [dram.tile([B, JK], BF16, name=f"arout{r}") for r in range(2)]
        rsout = dram.tile([B // NCORES, JK], BF16)
        wd = [dram.tile([B, JK], BF16, name=f"wd{r}") for r in (1, 2)]

        # resident tiles
        wres = wpool.tile([128, NCH, NOCT, JK], BF16, tag="wres")
        nc.sync.dma_start(wres[:, 0, 0:8], wl2_d[:, 0, 0:8])
        xT = xtp.tile([128, NCH, NOCT, B], BF16, tag="xT")

        def preload(ch, ocg):
            # stream the rest of W/xT behind the first compute-critical loads
            if ch == 0 and ocg == 1:
                nc.sync.dma_start(wres[:, 0, 8:16], wl2_d[:, 0, 8:16])
            elif ch == 0 and ocg == 2:
                nc.sync.dma_start(wres[:, 1, 0:8], wl2_d[:, 1, 0:8])
                nc.sync.dma_start(wres[:, 1, 8:16], wl2_d[:, 1, 8:16])
            elif ch == 0 and ocg == 3:
                nc.sync.dma_start(xT[:], xT2_d[:])
        u_hat = [uhp.tile([128, NSUB * BS, JK], BF16, tag=f"uh{c}",
                          name=f"u_hat{c}") for c in range(NCH)]

        # ---------------- P1: einsum + bounce; s0 between chunk passes ----
        # NOTE: GPSIMD/Pool cannot access PSUM on real HW -- drains must
        # stay on DVE/Act.
        drain_ch = [[nc.scalar, nc.vector], [nc.scalar, nc.vector]]
        s0ps = s0psum.tile([B, JK], F32, tag="s0ps")

        def einsum_ch(ch):
            for ocg in range(4):
                xbd = xpool.tile([128, 4, NSUB, 128], BF16, tag="xbd")
                nc.sync.dma_start(xbd[:], xbd2_d[:, ch, ocg * 4:(ocg + 1) * 4])
                preload(ch, ocg)
                for oo in range(4):
                    oc = ocg * 4 + oo
                    wt = wres[:, ch, oc, :]
                    tm4 = tmpp.tile([128, NSUB, JK], BF16, tag="tm4")
                    for half in range(2):
                        pe = epsum.tile([128, 2 * JK], F32, tag="pe")
                        for s2 in range(2):
                            sub = half * 2 + s2
                            nc.tensor.matmul(pe[:, s2 * JK:(s2 + 1) * JK],
                                             xbd[:, oo, sub, :], wt,
                                             start=True, stop=True,
                                             skip_group_check=True)
                        eng = drain_ch[ch][half]
                        dst = tm4[:, half * 2:(half + 1) * 2, :] \
                            .rearrange("p s f -> p (s f)")
                        if eng is nc.scalar:
                            eng.copy(dst, pe[:])
                        else:
                            eng.tensor_copy(dst, pe[:])
                    # bounce write: partitions (bs,il) at stride 32768
                    base = ub[:]
                    wdst = bass.AP(
                        tensor=base.tensor,
                        offset=(base.offset + ch * BS * 8 * 32768
                                + oc * NSUB * JK),
                        ap=[[NOCT * NSUB * JK, 128], [JK, NSUB], [1, JK]])
                    nc.sync.dma_start(wdst, tm4[:])

        def read_ch(ch, eng=None):
            # bounce read: partitions (il,oc) at stride 2048, per sub;
            # ch0 from the Act queue so SP can keep streaming ch1 writes,
            # ch1 from SP (runs right after its own writes)
            eng = eng or nc.scalar
            base = ub[:]
            for sub in range(NSUB):
                off = base.offset + ch * BS * 8 * 32768 + sub * JK
                src = bass.AP(tensor=base.tensor, offset=off,
                              ap=[[NSUB * JK, 128], [8 * 32768, BS], [1, JK]])
                eng.dma_start(
                    u_hat[ch][:, sub * BS:(sub + 1) * BS, :], src)

        einsum_ch(0)
        # s0 on PE after the ch0 einsum stream
        for ch in range(NCH):
            for oc in range(NOCT):
                nc.tensor.matmul(s0ps[:], xT[:, ch, oc, :], wres[:, ch, oc, :],
                                 start=(ch == 0 and oc == 0),
                                 stop=(ch == NCH - 1 and oc == NOCT - 1))
        read_ch(0)
        s0sb = vp.tile([B, JK], BF16, tag="s0sb")
        nc.scalar.mul(s0sb[:], s0ps[:], 1.0 / J)
        nc.scalar.dma_start(arin[0][:], s0sb[:])
        einsum_ch(1)

        def all_reduce(idx, last=False):
            if collectives:
                if last:
                    nc.gpsimd.collective_compute(
                        "ReduceScatter", OP.add,
                        replica_groups=[list(range(NCORES))],
                        ins=[arin[idx].opt()], outs=[rsout.opt()])
                else:
                    nc.gpsimd.collective_compute(
                        "AllReduce", OP.add,
                        replica_groups=[list(range(NCORES))],
                        ins=[arin[idx].opt()], outs=[arout[idx].opt()])
            else:
                if last:
                    nc.scalar.dma_start(rsout[:], arin[idx][:][0:B // NCORES])
                else:
                    nc.scalar.dma_start(arout[idx][:], arin[idx][:])

        all_reduce(0)

        # ---------------- rounds 1, 2 ----------------
        wacc = vp.tile([B, JK], F32, tag="wacc")

        def vchain(r):
            # squash previous round's s -> v_{r-1}; build w_r staging
            sv = vp.tile([B, JK], BF16, tag="sv")
            nc.scalar.dma_start(sv[:], arout[r - 1][:])
            v = _squash_emit(nc, vp, tiny, sv[:], BF16, tag=f"r{r}")
            if r == 1:
                nc.vector.tensor_copy(wacc[:], v[:])
                nc.scalar.dma_start(wd[0][:], v[:])
            else:
                nc.gpsimd.tensor_add(wacc[:], wacc[:], v[:])
                wb = vp.tile([B, JK], BF16, tag="wb")
                nc.vector.tensor_copy(wb[:], wacc[:])
                nc.scalar.dma_start(wd[1][:], wb[:])

        def round_body(r):
            wsrc = wd[r - 1][:]
            for g in range(NG8):
                # broadcast w[8b, 512] to all partitions
                wexp = wxp.tile([128, 8 * JK], BF16, tag="wexp")
                (nc.scalar if g == 0 else nc.gpsimd).dma_start(
                    wexp[:],
                    bass.AP(tensor=wsrc.tensor,
                            offset=wsrc.offset + g * 8 * JK,
                            ap=[[0, 128], [1, 8 * JK]]))
                pss = [spsum.tile([128, JK], F32, tag=f"ps{bq}",
                                  name=f"ps{r}_{g}_{bq}") for bq in range(2)]
                for ch in range(NCH):
                    uslc = u_hat[ch][:, g * 8:(g + 1) * 8, :] \
                        .rearrange("p b f -> p (b f)")
                    veng = nc.vector
                    prod = prp.tile([128, 8 * JK], BF16, tag="prod")
                    veng.tensor_mul(prod[:], uslc, wexp[:])
                    # k-reduction as a 2x-mode TT add tree (TensorReduce has
                    # no DVE perf modes -> 4x slower than this)
                    pv = prod[:].rearrange("p (bj k) -> p bj k", k=Kd)
                    t1 = smp.tile([128, 256, 8], BF16, tag="tr1")
                    veng.tensor_add(t1[:], pv[:, :, 0:8], pv[:, :, 8:16])
                    t2 = smp.tile([128, 256, 4], BF16, tag="tr2")
                    veng.tensor_add(t2[:], t1[:, :, 0:4], t1[:, :, 4:8])
                    t3 = smp.tile([128, 256, 2], BF16, tag="tr3")
                    veng.tensor_add(t3[:], t2[:, :, 0:2], t2[:, :, 2:4])
                    lg = smp.tile([128, 8, J], BF16, tag="lg")
                    veng.tensor_add(
                        lg[:].rearrange("p b j -> p (b j)"),
                        t3[:, :, 0], t3[:, :, 1])
                    # softmax, staged j-major so 1/z multiply stays 4x
                    est = smp.tile([128, J, 8], BF16, tag="est")
                    nc.scalar.activation(est[:].rearrange("p j b -> p b j"),
                                         lg[:], ACTF.Exp)
                    z = tiny.tile([128, 8], BF16, tag="z")
                    with nc.allow_low_precision("bf16 softmax z"):
                        nc.vector.tensor_reduce(
                            z[:], est[:].rearrange("p j b -> p b j"),
                            axis=AX.X, op=OP.add)
                    rz = tiny.tile([128, 8], BF16, tag="rz")
                    with nc.allow_low_precision("bf16 softmax 1/z"):
                        nc.vector.reciprocal(rz[:], z[:])
                    cst = smp.tile([128, J, 8], BF16, tag="cst")
                    nc.vector.tensor_mul(
                        cst[:], est[:],
                        rz[:, None, :].broadcast_to([128, J, 8]))
                    # c-weighted sums: 2 bq-groups x 4 col-tiled matmuls
                    for bq in range(2):
                        ps = pss[bq]
                        for b4 in range(4):
                            b = g * 8 + bq * 4 + b4
                            nc.tensor.matmul(
                                ps[b4 * 32:(b4 + 1) * 32, :],
                                cst[:, :, bq * 4 + b4],
                                u_hat[ch][:, b, :],
                                start=(ch == 0), stop=(ch == 1),
                                tile_position=(0, b4 * 32),
                                skip_group_check=True)
                        if ch == 1:
                            g2 = g * 2 + bq
                            sds = sdp.tile([128, JK], BF16, tag="sds")
                            nc.scalar.copy(sds[:], ps[:])
                            # pipelined staging + diag gather for this slice
                            nc.sync.dma_start(sstage[r - 1][g2], sds[:])
                            sb = sstage[r - 1][:]
                            diag = bass.AP(
                                tensor=sb.tensor,
                                offset=sb.offset + g2 * 128 * JK,
                                ap=[[32 * JK, 4], [JK + Kd, J], [1, Kd]])
                            ab = arin[r][:]
                            adst = bass.AP(
                                tensor=ab.tensor,
                                offset=ab.offset + (g * 8 + bq * 4) * JK,
                                ap=[[JK, 4], [Kd, J], [1, Kd]])
                            nc.sync.dma_start(adst, diag)

        vchain(1)
        read_ch(1, nc.sync)
        p1ctx.close()
        spsum = ctx.enter_context(tc.tile_pool(name="sp", bufs=3, space="PSUM"))
        wxp = ctx.enter_context(tc.tile_pool(name="wx", bufs=3))
        prp = ctx.enter_context(tc.tile_pool(name="pr", bufs=2))
        smp = ctx.enter_context(tc.tile_pool(name="smp", bufs=2))
        sdp = ctx.enter_context(tc.tile_pool(name="sdp", bufs=4))
        round_body(1)
        all_reduce(1)
        vchain(2)
        round_body(2)
        all_reduce(2, last=True)

        # final squash on this core's 8 batch rows
        svf = vp.tile([B // NCORES, JK], BF16, tag="svf")
        nc.scalar.dma_start(svf[:], rsout[:])
        vout = _squash_emit(nc, vp, tiny, svf[:], F32, nb=B // NCORES,
                            tag="fin")
        nc.scalar.dma_start(out_d[:].rearrange("b j k -> b (j k)"), vout[:])

    nc.compile()
    return nc


_NC_CACHE = None
_RUN_CACHE = None


def kernel(inputs, W, routings=3):
    """Full inputs in, full [B, J, K] output out. Shards over I across the
    8 NeuronCores internally; first call compiles and caches the executable."""
    global _NC_CACHE, _RUN_CACHE
    import jax
    from jax.sharding import NamedSharding, PartitionSpec
    inputs = np.asarray(inputs, dtype=np.float32)
    W = np.asarray(W, dtype=np.float32)
    if _NC_CACHE is None:
        _NC_CACHE = build_program()
    nc = _NC_CACHE
    if _RUN_CACHE is None:
        _RUN_CACHE = _build_sharded(nc)
    fn, mesh, in_names, out_names, out_avals, zero_outs = _RUN_CACHE
    per_core = [_host_prep(inputs, W, core) for core in range(NCORES)]
    sh = NamedSharding(mesh, PartitionSpec("core"))
    concat_in = [jax.device_put(
        np.concatenate([per_core[c][n] for c in range(NCORES)], axis=0), sh)
        for n in in_names]
    zeros = [jax.device_put(
        np.zeros((NCORES * z.shape[0], *z.shape[1:]), z.dtype), sh)
        for z in zero_outs]
    out = fn(*concat_in, *zeros)
    jax.block_until_ready(out)
    oidx = out_names.index("out")
    return np.asarray(out[oidx]).reshape(B, J, Kd)


# ---------------- timing harness (test-only) ----------------
def _build_sharded(nc):
    """Replicate bass2jax.run_bass_via_pjrt's jit construction, returning
    (fn, mesh, in_names, out_names, out_avals, zero_outs)."""
    import jax
    from jax.sharding import Mesh, PartitionSpec
    from jax.experimental.shard_map import shard_map
    from concourse.bass2jax import (_bass_exec_p, install_neuronx_cc_hook,
                                    partition_id_tensor)
    install_neuronx_cc_hook()
    partition_name = (nc.partition_id_tensor.name
                      if nc.partition_id_tensor else None)
    in_names, out_names, out_avals, zero_outs = [], [], [], []
    for alloc in nc.m.functions[0].allocations:
        if not isinstance(alloc, mybir.MemoryLocationSet):
            continue
        name = alloc.memorylocations[0].name
        if alloc.kind == "ExternalInput":
            if name != partition_name:
                in_names.append(name)
        elif alloc.kind == "ExternalOutput":
            out_names.append(name)
            shape = tuple(alloc.tensor_shape)
            dtype = mybir.dt.np(alloc.dtype)
            out_avals.append(jax.core.ShapedArray(shape, dtype))
            zero_outs.append(np.zeros(shape, dtype))
    n_params = len(in_names)
    n_outs = len(out_avals)
    all_in = list(in_names) + list(out_names)
    if partition_name is not None:
        all_in.append(partition_name)
    donate = tuple(range(n_params, n_params + n_outs))

    def _body(*args):
        operands = list(args)
        if partition_name is not None:
            operands.append(partition_id_tensor())
        return tuple(_bass_exec_p.bind(
            *operands, out_avals=tuple(out_avals), in_names=tuple(all_in),
            out_names=tuple(out_names), lowering_input_output_aliases=(),
            sim_require_finite=True, sim_require_nnan=True, nc=nc))

    devices = jax.devices()[:NCORES]
    mesh = Mesh(np.array(devices), ("core",))
    in_specs = (PartitionSpec("core"),) * (n_params + n_outs)
    out_specs = (PartitionSpec("core"),) * n_outs
    fn = jax.jit(shard_map(_body, mesh=mesh, in_specs=in_specs,
                           out_specs=out_specs, check_rep=False),
                 donate_argnums=donate, keep_unused=True)
    return fn, mesh, in_names[:n_params], out_names, out_avals, zero_outs


def timed_run(inputs, W, iters=20):
    """Returns (best_ns, times_ns list, output)."""
    import time, jax
    from jax.sharding import NamedSharding, PartitionSpec
    global _NC_CACHE, _RUN_CACHE
    if _NC_CACHE is None:
        _NC_CACHE = build_program()
    nc = _NC_CACHE
    if _RUN_CACHE is None:
        _RUN_CACHE = _build_sharded(nc)
    fn, mesh, in_names, out_names, out_avals, zero_outs = _RUN_CACHE
    per_core = [_host_prep(inputs, W, core) for core in range(NCORES)]
    sh = NamedSharding(mesh, PartitionSpec("core"))
    concat_in = [jax.device_put(
        np.concatenate([per_core[c][n] for c in range(NCORES)], axis=0), sh)
        for n in in_names]
    def make_zeros():
        return [jax.device_put(
            np.zeros((NCORES * z.shape[0], *z.shape[1:]), z.dtype), sh)
            for z in zero_outs]
    zsets = [make_zeros() for _ in range(iters + 3)]
    out = None
    times = []
    for it in range(iters + 3):
        t0 = time.perf_counter_ns()
        res = fn(*concat_in, *zsets[it])
        jax.block_until_ready(res)
        dt = time.perf_counter_ns() - t0
        if it >= 3:
            times.append(dt)
        out = res
    oidx = out_names.index("out")
    out_np = np.asarray(out[oidx]).reshape(B, J, Kd)
    return min(times), times, out_np


# revision 64
# speedup vs baseline: 1.5839x; 1.0017x over previous
"""CapsuleLayer (dynamic routing) Trainium2 kernel — 8 NeuronCores.

Strategy: shard over input capsules I (2048 -> 256/core); W load 4 MB/core
(bf16). Routing state is per-(b, i, j) and core-local; the three routing
reductions s_r are per-core PE partials, AllReduce'd in bf16 (64 KB).

Per-core pipeline (ordered to keep the in-order engine/DMA queues clear):
  P1  u_hat einsum: block-diag(x) [128,128] @ W-octet [128,512] matmuls,
      two N=512 col-tiled matmuls per 2-bank psum tile; psum drained as
      [128,1024] bf16 copies (DVE+Act only -- GPSIMD cannot touch PSUM);
      DRAM bounce with 3-dim APs on both sides (write partitions (bs,il)
      stride 32768, read partitions (il,oc) stride 2048) -> resident
      u_hat[ch] [i-part, (b, jk)] bf16, 128 KiB/partition.
      s0 = (1/J) sum_i u_hat as 32 direct xT @ W matmuls emitted between
      the two einsum chunk passes; its AllReduce + squash(v0) + w-staging
      complete while the bounce streams.
  P2  rounds 1,2 (8 groups of 8 batch x 2 i-chunks):
      logits = reduce_k(u_hat * w_bcast): TT multiply in 2x bf16 mode +
      a 4-level TT add tree (TensorReduce has no DVE perf modes and would
      be 4x slower); softmax staged j-major so the 1/z multiply stays in
      packed mode; c-weighted sums as 4 col-tiled PE matmuls per 4-batch
      sharing one psum tile across both i-chunks; per-slice staging DMA +
      7ns/descriptor diagonal gather DRAM->DRAM, pipelined into the round;
      AllReduce (r1) / ReduceScatter (r2) -> squash -> [8, 512] out.
      DMA issue is spread across queues: SP = bounce + staging + wexp-free
      slots, Act = bounce reads (ch0) + v-chain, Pool/SWDGE = wexp
      broadcasts, so no queue head-of-line-blocks another phase.
"""
import numpy as np
import ml_dtypes
from contextlib import ExitStack

import concourse.bass as bass
import concourse.mybir as mybir
import concourse.tile as tile
from concourse import bacc
from concourse import bass_utils

B, I, D, J, Kd = 64, 2048, 16, 32, 16
NCORES = 8
IC = I // NCORES      # 256 input capsules per core
NCH = 2               # i-chunks of 128 per core
NOCT = 16             # octets of 8 i per chunk
NSUB = 4              # sub-batches of b
BS = B // NSUB        # 16
JK = J * Kd           # 512
NG8 = 8               # groups of 8 batch elements
EPS = 1e-7
BF16 = mybir.dt.bfloat16
F32 = mybir.dt.float32
AX = mybir.AxisListType
OP = mybir.AluOpType
ACTF = mybir.ActivationFunctionType


def _host_prep(inputs, W, core):
    """Per-core DMA-ready layouts (bf16)."""
    bf = ml_dtypes.bfloat16
    Wc = W[core * IC:(core + 1) * IC]  # [256, 32, 16, 16] = [i, j, d, k]
    # wl2[(il,d), ch, oc, (j,k)]
    wl2 = Wc.reshape(NCH, NOCT, 8, J, D, Kd).transpose(2, 4, 0, 1, 3, 5) \
            .reshape(128, NCH, NOCT, JK)
    xc = inputs[:, core * IC:(core + 1) * IC, :]  # [64, 256, 16] = [b, i, d]
    xr = xc.reshape(B, NCH, NOCT, 8, D)           # [b, ch, oc, il, d]
    # xbd2[(il,d), ch, oc, sub, bs*8+il] = x[sub*16+bs, i(ch,oc,il), d]
    xbd2 = np.zeros((128, NCH, NOCT, NSUB, 128), np.float32)
    for il in range(8):
        blk = xr[:, :, :, il, :].reshape(NSUB, BS, NCH, NOCT, D) \
                                .transpose(4, 2, 3, 0, 1)
        xbd2[il * 16:(il + 1) * 16, :, :, :, il::8] = blk
    # xT2[(il,d), ch, oc, b]
    xT2 = np.ascontiguousarray(xr.transpose(3, 4, 1, 2, 0)).reshape(
        128, NCH, NOCT, B)
    return {"wl2": np.ascontiguousarray(wl2).astype(bf),
            "xbd2": xbd2.astype(bf),
            "xT2": np.ascontiguousarray(xT2).astype(bf)}


def _squash_emit(nc, pool, tiny, src_ap, out_dtype, nb=B, tag=""):
    """squash on s [nb, 512] viewed [nb, 32, 16]; returns v tile [nb, 512]."""
    sq = pool.tile([nb, JK], BF16, tag="sq")
    nc.vector.tensor_mul(sq[:], src_ap, src_ap)
    nn = tiny.tile([nb, J], F32, tag="nn")
    nc.vector.tensor_reduce(nn[:], sq[:].rearrange("b (j k) -> b j k", k=Kd),
                            axis=AX.X, op=OP.add)
    t1 = tiny.tile([nb, J], F32, tag="t1")
    nc.vector.tensor_scalar_add(t1[:], nn[:], 1.0)
    t2 = tiny.tile([nb, J], F32, tag="t2")
    nc.vector.tensor_scalar_add(t2[:], nn[:], EPS)
    st = tiny.tile([nb, J], F32, tag="st")
    nc.scalar.sqrt(st[:], t2[:])
    den = tiny.tile([nb, J], F32, tag="den")
    nc.vector.tensor_mul(den[:], t1[:], st[:])
    rden = tiny.tile([nb, J], F32, tag="rden")
    nc.vector.reciprocal(rden[:], den[:])
    sc = tiny.tile([nb, J], F32, tag="sc")
    nc.vector.tensor_mul(sc[:], nn[:], rden[:])
    v = pool.tile([nb, JK], out_dtype, tag="vout" + ("f" if out_dtype == F32 else ""))
    nc.vector.tensor_mul(
        v[:].rearrange("b (j k) -> b j k", k=Kd),
        src_ap.rearrange("b (j k) -> b j k", k=Kd),
        sc[:, :, None].broadcast_to([nb, J, Kd]))
    return v


def build_program(collectives=True):
    nc = bacc.Bacc("TRN2", target_bir_lowering=False, debug=False,
                   num_devices=NCORES if collectives else 1)
    wl2_d = nc.dram_tensor("wl2", [128, NCH, NOCT, JK], BF16,
                           kind="ExternalInput")
    xbd2_d = nc.dram_tensor("xbd2", [128, NCH, NOCT, NSUB, 128], BF16,
                            kind="ExternalInput")
    xT2_d = nc.dram_tensor("xT2", [128, NCH, NOCT, B], BF16,
                           kind="ExternalInput")
    out_d = nc.dram_tensor("out", [B // NCORES, J, Kd], F32,
                           kind="ExternalOutput")

    with tile.TileContext(nc) as tc, ExitStack() as ctx:
        dram = ctx.enter_context(tc.tile_pool(name="dram", bufs=1, space="DRAM"))
        uhp = ctx.enter_context(tc.tile_pool(name="uh", bufs=1))
        tiny = ctx.enter_context(tc.tile_pool(name="ty", bufs=3))
        vp = ctx.enter_context(tc.tile_pool(name="vp", bufs=1))
        p1ctx = ExitStack()
        wpool = p1ctx.enter_context(tc.tile_pool(name="wp", bufs=1))
        xpool = p1ctx.enter_context(tc.tile_pool(name="xp", bufs=2))
        xtp = p1ctx.enter_context(tc.tile_pool(name="xtp", bufs=1))
        epsum = p1ctx.enter_context(tc.tile_pool(name="ep", bufs=3, space="PSUM"))
        s0psum = p1ctx.enter_context(tc.tile_pool(name="s0p", bufs=1, space="PSUM"))
        tmpp = p1ctx.enter_context(tc.tile_pool(name="tm", bufs=4))

        # bounce layout: addr = ((ch*16+bs)*8+il)*32768 + oc*2048 + sub*512 + e
        # write partitions (bs,il) stride 32768; read partitions (il,oc)
        # stride 2048 -> both sides are 3-dim APs.
        ub = dram.tile([NCH, BS, 8, NOCT, NSUB, JK], BF16)
        sstage = [dram.tile([2 * NG8, 128, JK], BF16, name=f"sst{r}")
                  for r in (1, 2)]
        arin = [dram.tile([B, JK], BF16, name=f"arin{r}") for r in range(3)]
        arout = [dram.tile([B, JK], BF16, name=f"arout{r}") for r in range(2)]
        rsout = dram.tile([B // NCORES, JK], BF16)
        wd = [dram.tile([B, JK], BF16, name=f"wd{r}") for r in (1, 2)]

        # resident tiles
        wres = wpool.tile([128, NCH, NOCT, JK], BF16, tag="wres")
        nc.sync.dma_start(wres[:, 0, 0:8], wl2_d[:, 0, 0:8])
        xT = xtp.tile([128, NCH, NOCT, B], BF16, tag="xT")

        def preload(ch, ocg):
            # stream the rest of W/xT behind the first compute-critical loads
            if ch == 0 and ocg == 1:
                nc.sync.dma_start(wres[:, 0, 8:16], wl2_d[:, 0, 8:16])
            elif ch == 0 and ocg == 2:
                nc.sync.dma_start(wres[:, 1, 0:8], wl2_d[:, 1, 0:8])
                nc.sync.dma_start(wres[:, 1, 8:16], wl2_d[:, 1, 8:16])
            elif ch == 0 and ocg == 3:
                nc.sync.dma_start(xT[:], xT2_d[:])
        u_hat = [uhp.tile([128, NSUB * BS, JK], BF16, tag=f"uh{c}",
                          name=f"u_hat{c}") for c in range(NCH)]

        # ---------------- P1: einsum + bounce; s0 between chunk passes ----
        # NOTE: GPSIMD/Pool cannot access PSUM on real HW -- drains must
        # stay on DVE/Act.
        drain_ch = [[nc.scalar, nc.vector], [nc.scalar, nc.vector]]
        s0ps = s0psum.tile([B, JK], F32, tag="s0ps")

        def einsum_ch(ch):
            for ocg in range(4):
                xbd = xpool.tile([128, 4, NSUB, 128], BF16, tag="xbd")
                nc.sync.dma_start(xbd[:], xbd2_d[:, ch, ocg * 4:(ocg + 1) * 4])
                preload(ch, ocg)
                for oo in range(4):
                    oc = ocg * 4 + oo
                    wt = wres[:, ch, oc, :]
                    tm4 = tmpp.tile([128, NSUB, JK], BF16, tag="tm4")
                    for half in range(2):
                        pe = epsum.tile([128, 2 * JK], F32, tag="pe")
                        for s2 in range(2):
                            sub = half * 2 + s2
                            nc.tensor.matmul(pe[:, s2 * JK:(s2 + 1) * JK],
                                             xbd[:, oo, sub, :], wt,
                                             start=True, stop=True,
                                             skip_group_check=True)
                        eng = drain_ch[ch][half]
                        dst = tm4[:, half * 2:(half + 1) * 2, :] \
                            .rearrange("p s f -> p (s f)")
                        if eng is nc.scalar:
                            eng.copy(dst, pe[:])
                        else:
                            eng.tensor_copy(dst, pe[:])
                    # bounce write: partitions (bs,il) at stride 32768
                    base = ub[:]
                    wdst = bass.AP(
                        tensor=base.tensor,
                        offset=(base.offset + ch * BS * 8 * 32768
                                + oc * NSUB * JK),
                        ap=[[NOCT * NSUB * JK, 128], [JK, NSUB], [1, JK]])
                    nc.sync.dma_start(wdst, tm4[:])

        def read_ch(ch, eng=None):
            # bounce read: partitions (il,oc) at stride 2048, per sub;
            # ch0 from the Act queue so SP can keep streaming ch1 writes,
            # ch1 from SP (runs right after its own writes)
            eng = eng or nc.scalar
            base = ub[:]
            for sub in range(NSUB):
                off = base.offset + ch * BS * 8 * 32768 + sub * JK
                src = bass.AP(tensor=base.tensor, offset=off,
                              ap=[[NSUB * JK, 128], [8 * 32768, BS], [1, JK]])
                eng.dma_start(
                    u_hat[ch][:, sub * BS:(sub + 1) * BS, :], src)

        einsum_ch(0)
        # s0 on PE after the ch0 einsum stream
        for ch in range(NCH):
            for oc in range(NOCT):
                nc.tensor.matmul(s0ps[:], xT[:, ch, oc, :], wres[:, ch, oc, :],
                                 start=(ch == 0 and oc == 0),
                                 stop=(ch == NCH - 1 and oc == NOCT - 1))
        read_ch(0)
        s0sb = vp.tile([B, JK], BF16, tag="s0sb")
        nc.scalar.mul(s0sb[:], s0ps[:], 1.0 / J)
        nc.scalar.dma_start(arin[0][:], s0sb[:])
        einsum_ch(1)

        def all_reduce(idx, last=False):
            if collectives:
                if last:
                    nc.gpsimd.collective_compute(
                        "ReduceScatter", OP.add,
                        replica_groups=[list(range(NCORES))],
                        ins=[arin[idx].opt()], outs=[rsout.opt()])
                else:
                    nc.gpsimd.collective_compute(
                        "AllReduce", OP.add,
                        replica_groups=[list(range(NCORES))],
                        ins=[arin[idx].opt()], outs=[arout[idx].opt()])
            else:
                if last:
                    nc.scalar.dma_start(rsout[:], arin[idx][:][0:B // NCORES])
                else:
                    nc.scalar.dma_start(arout[idx][:], arin[idx][:])

        all_reduce(0)

        # ---------------- rounds 1, 2 ----------------
        wacc = vp.tile([B, JK], F32, tag="wacc")

        def vchain(r):
            # squash previous round's s -> v_{r-1}; build w_r staging
            sv = vp.tile([B, JK], BF16, tag="sv")
            nc.scalar.dma_start(sv[:], arout[r - 1][:])
            v = _squash_emit(nc, vp, tiny, sv[:], BF16, tag=f"r{r}")
            if r == 1:
                nc.vector.tensor_copy(wacc[:], v[:])
                nc.scalar.dma_start(wd[0][:], v[:])
            else:
                nc.gpsimd.tensor_add(wacc[:], wacc[:], v[:])
                wb = vp.tile([B, JK], BF16, tag="wb")
                nc.vector.tensor_copy(wb[:], wacc[:])
                nc.scalar.dma_start(wd[1][:], wb[:])

        def round_body(r):
            wsrc = wd[r - 1][:]
            for g in range(NG8):
                # broadcast w[8b, 512] to all partitions
                wexp = wxp.tile([128, 8 * JK], BF16, tag="wexp")
                (nc.scalar if g == 0 else nc.gpsimd).dma_start(
                    wexp[:],
                    bass.AP(tensor=wsrc.tensor,
                            offset=wsrc.offset + g * 8 * JK,
                            ap=[[0, 128], [1, 8 * JK]]))
                pss = [spsum.tile([128, JK], F32, tag=f"ps{bq}",
                                  name=f"ps{r}_{g}_{bq}") for bq in range(2)]
                for ch in range(NCH):
                    uslc = u_hat[ch][:, g * 8:(g + 1) * 8, :] \
                        .rearrange("p b f -> p (b f)")
                    veng = nc.vector
                    prod = prp.tile([128, 8 * JK], BF16, tag="prod")
                    veng.tensor_mul(prod[:], uslc, wexp[:])
                    # k-reduction as a 2x-mode TT add tree (TensorReduce has
                    # no DVE perf modes -> 4x slower than this)
                    pv = prod[:].rearrange("p (bj k) -> p bj k", k=Kd)
                    t1 = smp.tile([128, 256, 8], BF16, tag="tr1")
                    veng.tensor_add(t1[:], pv[:, :, 0:8], pv[:, :, 8:16])
                    t2 = smp.tile([128, 256, 4], BF16, tag="tr2")
                    veng.tensor_add(t2[:], t1[:, :, 0:4], t1[:, :, 4:8])
                    t3 = smp.tile([128, 256, 2], BF16, tag="tr3")
                    veng.tensor_add(t3[:], t2[:, :, 0:2], t2[:, :, 2:4])
                    lg = smp.tile([128, 8, J], BF16, tag="lg")
                    veng.tensor_add(
                        lg[:].rearrange("p b j -> p (b j)"),
                        t3[:, :, 0], t3[:, :, 1])
                    # softmax, staged j-major so 1/z multiply stays 4x
                    est = smp.tile([128, J, 8], BF16, tag="est")
                    nc.scalar.activation(est[:].rearrange("p j b -> p b j"),
                                         lg[:], ACTF.Exp)
                    z = tiny.tile([128, 8], BF16, tag="z")
                    with nc.allow_low_precision("bf16 softmax z"):
                        nc.vector.tensor_reduce(
                            z[:], est[:].rearrange("p j b -> p b j"),
                            axis=AX.X, op=OP.add)
                    rz = tiny.tile([128, 8], BF16, tag="rz")
                    with nc.allow_low_precision("bf16 softmax 1/z"):
                        nc.vector.reciprocal(rz[:], z[:])
                    cst = smp.tile([128, J, 8], BF16, tag="cst")
                    nc.vector.tensor_mul(
                        cst[:], est[:],
                        rz[:, None, :].broadcast_to([128, J, 8]))
                    # c-weighted sums: 2 bq-groups x 4 col-tiled matmuls
                    for bq in range(2):
                        ps = pss[bq]
                        for b4 in range(4):
                            b = g * 8 + bq * 4 + b4
                            nc.tensor.matmul(
                                ps[b4 * 32:(b4 + 1) * 32, :],
                                cst[:, :, bq * 4 + b4],
                                u_hat[ch][:, b, :],
                                start=(ch == 0), stop=(ch == 1),
                                tile_position=(0, b4 * 32),
                                skip_group_check=True)
                        if ch == 1:
                            g2 = g * 2 + bq
                            sds = sdp.tile([128, JK], BF16, tag="sds")
                            nc.scalar.copy(sds[:], ps[:])
                            # pipelined staging + diag gather for this slice
                            nc.sync.dma_start(sstage[r - 1][g2], sds[:])
                            sb = sstage[r - 1][:]
                            diag = bass.AP(
                                tensor=sb.tensor,
                                offset=sb.offset + g2 * 128 * JK,
                                ap=[[32 * JK, 4], [JK + Kd, J], [1, Kd]])
                            ab = arin[r][:]
                            adst = bass.AP(
                                tensor=ab.tensor,
                                offset=ab.offset + (g * 8 + bq * 4) * JK,
                                ap=[[JK, 4], [Kd, J], [1, Kd]])
                            nc.sync.dma_start(adst, diag)

        vchain(1)
        read_ch(1, nc.sync)
        p1ctx.close()
        spsum = ctx.enter_context(tc.tile_pool(name="sp", bufs=3, space="PSUM"))
        wxp = ctx.enter_context(tc.tile_pool(name="wx", bufs=3))
        prp = ctx.enter_context(tc.tile_pool(name="pr", bufs=2))
        smp = ctx.enter_context(tc.tile_pool(name="smp", bufs=2))
        sdp = ctx.enter_context(tc.tile_pool(name="sdp", bufs=6))
        round_body(1)
        all_reduce(1)
        vchain(2)
        round_body(2)
        all_reduce(2, last=True)

        # final squash on this core's 8 batch rows
        svf = vp.tile([B // NCORES, JK], BF16, tag="svf")
        nc.scalar.dma_start(svf[:], rsout[:])
        vout = _squash_emit(nc, vp, tiny, svf[:], F32, nb=B // NCORES,
                            tag="fin")
        nc.scalar.dma_start(out_d[:].rearrange("b j k -> b (j k)"), vout[:])

    nc.compile()
    return nc


_NC_CACHE = None
_RUN_CACHE = None


def kernel(inputs, W, routings=3):
    """Full inputs in, full [B, J, K] output out. Shards over I across the
    8 NeuronCores internally; first call compiles and caches the executable."""
    global _NC_CACHE, _RUN_CACHE
    import jax
    from jax.sharding import NamedSharding, PartitionSpec
    inputs = np.asarray(inputs, dtype=np.float32)
    W = np.asarray(W, dtype=np.float32)
    if _NC_CACHE is None:
        _NC_CACHE = build_program()
    nc = _NC_CACHE
    if _RUN_CACHE is None:
        _RUN_CACHE = _build_sharded(nc)
    fn, mesh, in_names, out_names, out_avals, zero_outs = _RUN_CACHE
    per_core = [_host_prep(inputs, W, core) for core in range(NCORES)]
    sh = NamedSharding(mesh, PartitionSpec("core"))
    concat_in = [jax.device_put(
        np.concatenate([per_core[c][n] for c in range(NCORES)], axis=0), sh)
        for n in in_names]
    zeros = [jax.device_put(
        np.zeros((NCORES * z.shape[0], *z.shape[1:]), z.dtype), sh)
        for z in zero_outs]
    out = fn(*concat_in, *zeros)
    jax.block_until_ready(out)
    oidx = out_names.index("out")
    return np.asarray(out[oidx]).reshape(B, J, Kd)


# ---------------- timing harness (test-only) ----------------
def _build_sharded(nc):
    """Replicate bass2jax.run_bass_via_pjrt's jit construction, returning
    (fn, mesh, in_names, out_names, out_avals, zero_outs)."""
    import jax
    from jax.sharding import Mesh, PartitionSpec
    from jax.experimental.shard_map import shard_map
    from concourse.bass2jax import (_bass_exec_p, install_neuronx_cc_hook,
                                    partition_id_tensor)
    install_neuronx_cc_hook()
    partition_name = (nc.partition_id_tensor.name
                      if nc.partition_id_tensor else None)
    in_names, out_names, out_avals, zero_outs = [], [], [], []
    for alloc in nc.m.functions[0].allocations:
        if not isinstance(alloc, mybir.MemoryLocationSet):
            continue
        name = alloc.memorylocations[0].name
        if alloc.kind == "ExternalInput":
            if name != partition_name:
                in_names.append(name)
        elif alloc.kind == "ExternalOutput":
            out_names.append(name)
            shape = tuple(alloc.tensor_shape)
            dtype = mybir.dt.np(alloc.dtype)
            out_avals.append(jax.core.ShapedArray(shape, dtype))
            zero_outs.append(np.zeros(shape, dtype))
    n_params = len(in_names)
    n_outs = len(out_avals)
    all_in = list(in_names) + list(out_names)
    if partition_name is not None:
        all_in.append(partition_name)
    donate = tuple(range(n_params, n_params + n_outs))

    def _body(*args):
        operands = list(args)
        if partition_name is not None:
            operands.append(partition_id_tensor())
        return tuple(_bass_exec_p.bind(
            *operands, out_avals=tuple(out_avals), in_names=tuple(all_in),
            out_names=tuple(out_names), lowering_input_output_aliases=(),
            sim_require_finite=True, sim_require_nnan=True, nc=nc))

    devices = jax.devices()[:NCORES]
    mesh = Mesh(np.array(devices), ("core",))
    in_specs = (PartitionSpec("core"),) * (n_params + n_outs)
    out_specs = (PartitionSpec("core"),) * n_outs
    fn = jax.jit(shard_map(_body, mesh=mesh, in_specs=in_specs,
                           out_specs=out_specs, check_rep=False),
                 donate_argnums=donate, keep_unused=True)
    return fn, mesh, in_names[:n_params], out_names, out_avals, zero_outs


def timed_run(inputs, W, iters=20):
    """Returns (best_ns, times_ns list, output)."""
    import time, jax
    from jax.sharding import NamedSharding, PartitionSpec
    global _NC_CACHE, _RUN_CACHE
    if _NC_CACHE is None:
        _NC_CACHE = build_program()
    nc = _NC_CACHE
    if _RUN_CACHE is None:
        _RUN_CACHE = _build_sharded(nc)
    fn, mesh, in_names, out_names, out_avals, zero_outs = _RUN_CACHE
    per_core = [_host_prep(inputs, W, core) for core in range(NCORES)]
    sh = NamedSharding(mesh, PartitionSpec("core"))
    concat_in = [jax.device_put(
        np.concatenate([per_core[c][n] for c in range(NCORES)], axis=0), sh)
        for n in in_names]
    def make_zeros():
        return [jax.device_put(
            np.zeros((NCORES * z.shape[0], *z.shape[1:]), z.dtype), sh)
            for z in zero_outs]
    zsets = [make_zeros() for _ in range(iters + 3)]
    out = None
    times = []
    for it in range(iters + 3):
        t0 = time.perf_counter_ns()
        res = fn(*concat_in, *zsets[it])
        jax.block_until_ready(res)
        dt = time.perf_counter_ns() - t0
        if it >= 3:
            times.append(dt)
        out = res
    oidx = out_names.index("out")
    out_np = np.asarray(out[oidx]).reshape(B, J, Kd)
    return min(times), times, out_np


# revision 68
# speedup vs baseline: 1.7181x; 1.0847x over previous
"""CapsuleLayer (dynamic routing) Trainium2 kernel — 8 NeuronCores.

Strategy: shard over input capsules I (2048 -> 256/core); W load 4 MB/core
(bf16). Routing state is per-(b, i, j) and core-local; the three routing
reductions s_r are per-core PE partials, AllReduce'd in bf16 (64 KB).

Per-core pipeline (ordered to keep the in-order engine/DMA queues clear):
  P1  u_hat einsum: block-diag(x) [128,128] @ W-octet [128,512] matmuls,
      two N=512 col-tiled matmuls per 2-bank psum tile; psum drained as
      [128,1024] bf16 copies (DVE+Act only -- GPSIMD cannot touch PSUM);
      DRAM bounce with 3-dim APs on both sides (write partitions (bs,il)
      stride 32768, read partitions (il,oc) stride 2048) -> resident
      u_hat[ch] [i-part, (b, jk)] bf16, 128 KiB/partition.
      s0 = (1/J) sum_i u_hat as 32 direct xT @ W matmuls emitted between
      the two einsum chunk passes; its AllReduce + squash(v0) + w-staging
      complete while the bounce streams.
  P2  rounds 1,2 (8 groups of 8 batch x 2 i-chunks):
      logits = reduce_k(u_hat * w_bcast): TT multiply in 2x bf16 mode +
      a 4-level TT add tree (TensorReduce has no DVE perf modes and would
      be 4x slower); softmax staged j-major so the 1/z multiply stays in
      packed mode; c-weighted sums as 4 col-tiled PE matmuls per 4-batch
      sharing one psum tile across both i-chunks; per-slice staging DMA +
      7ns/descriptor diagonal gather DRAM->DRAM, pipelined into the round;
      AllReduce (r1) / ReduceScatter (r2) -> squash -> [8, 512] out.
      DMA issue is spread across queues: SP = bounce + staging + wexp-free
      slots, Act = bounce reads (ch0) + v-chain, Pool/SWDGE = wexp
      broadcasts, so no queue head-of-line-blocks another phase.
"""
import numpy as np
import ml_dtypes
from contextlib import ExitStack

import concourse.bass as bass
import concourse.mybir as mybir
import concourse.tile as tile
from concourse import bacc
from concourse import bass_utils

B, I, D, J, Kd = 64, 2048, 16, 32, 16
NCORES = 8
IC = I // NCORES      # 256 input capsules per core
NCH = 2               # i-chunks of 128 per core
NOCT = 16             # octets of 8 i per chunk
NSUB = 4              # sub-batches of b
BS = B // NSUB        # 16
JK = J * Kd           # 512
NG8 = 8               # groups of 8 batch elements
EPS = 1e-7
BF16 = mybir.dt.bfloat16
F32 = mybir.dt.float32
AX = mybir.AxisListType
OP = mybir.AluOpType
ACTF = mybir.ActivationFunctionType


def _host_prep(inputs, W, core):
    """Per-core DMA-ready layouts (bf16)."""
    bf = ml_dtypes.bfloat16
    Wc = W[core * IC:(core + 1) * IC]  # [256, 32, 16, 16] = [i, j, d, k]
    # wl2[(il,d), ch, oc, (j,k)]
    wl2 = Wc.reshape(NCH, NOCT, 8, J, D, Kd).transpose(2, 4, 0, 1, 3, 5) \
            .reshape(128, NCH, NOCT, JK)
    xc = inputs[:, core * IC:(core + 1) * IC, :]  # [64, 256, 16] = [b, i, d]
    xr = xc.reshape(B, NCH, NOCT, 8, D)           # [b, ch, oc, il, d]
    # xbd2[(il,d), ch, oc, sub, bs*8+il] = x[sub*16+bs, i(ch,oc,il), d]
    xbd2 = np.zeros((128, NCH, NOCT, NSUB, 128), np.float32)
    for il in range(8):
        blk = xr[:, :, :, il, :].reshape(NSUB, BS, NCH, NOCT, D) \
                                .transpose(4, 2, 3, 0, 1)
        xbd2[il * 16:(il + 1) * 16, :, :, :, il::8] = blk
    # xT2[(il,d), ch, oc, b]
    xT2 = np.ascontiguousarray(xr.transpose(3, 4, 1, 2, 0)).reshape(
        128, NCH, NOCT, B)
    return {"wl2": np.ascontiguousarray(wl2).astype(bf),
            "xbd2": xbd2.astype(bf),
            "xT2": np.ascontiguousarray(xT2).astype(bf)}


def _squash_emit(nc, pool, tiny, src_ap, out_dtype, nb=B, tag="", off=0):
    """squash on s [nb, 512] viewed [nb, 32, 16]; returns v tile.
    off: base partition for every tile (HW: TT inputs need equal bases)."""
    an = off + nb
    s_ = slice(off, an)
    sq = pool.tile([an, JK], BF16, tag="sq")
    nc.vector.tensor_mul(sq[s_], src_ap, src_ap)
    nn = tiny.tile([an, J], F32, tag="nn")
    nc.vector.tensor_reduce(nn[s_], sq[s_].rearrange("b (j k) -> b j k", k=Kd),
                            axis=AX.X, op=OP.add)
    t1 = tiny.tile([an, J], F32, tag="t1")
    nc.vector.tensor_scalar_add(t1[s_], nn[s_], 1.0)
    t2 = tiny.tile([an, J], F32, tag="t2")
    nc.vector.tensor_scalar_add(t2[s_], nn[s_], EPS)
    st = tiny.tile([an, J], F32, tag="st")
    nc.scalar.sqrt(st[s_], t2[s_])
    den = tiny.tile([an, J], F32, tag="den")
    nc.vector.tensor_mul(den[s_], t1[s_], st[s_])
    rden = tiny.tile([an, J], F32, tag="rden")
    nc.vector.reciprocal(rden[s_], den[s_])
    sc = tiny.tile([an, J], F32, tag="sc")
    nc.vector.tensor_mul(sc[s_], nn[s_], rden[s_])
    v = pool.tile([an, JK], out_dtype,
                  tag="vout" + ("f" if out_dtype == F32 else ""))
    nc.vector.tensor_mul(
        v[s_].rearrange("b (j k) -> b j k", k=Kd),
        src_ap.rearrange("b (j k) -> b j k", k=Kd),
        sc[s_][:, :, None].broadcast_to([nb, J, Kd]))
    return v[s_]


def build_program(collectives=True):
    nc = bacc.Bacc("TRN2", target_bir_lowering=False, debug=False,
                   num_devices=NCORES if collectives else 1)
    wl2_d = nc.dram_tensor("wl2", [128, NCH, NOCT, JK], BF16,
                           kind="ExternalInput")
    xbd2_d = nc.dram_tensor("xbd2", [128, NCH, NOCT, NSUB, 128], BF16,
                            kind="ExternalInput")
    xT2_d = nc.dram_tensor("xT2", [128, NCH, NOCT, B], BF16,
                           kind="ExternalInput")
    out_d = nc.dram_tensor("out", [B // NCORES, J, Kd], F32,
                           kind="ExternalOutput")

    with tile.TileContext(nc) as tc, ExitStack() as ctx:
        dram = ctx.enter_context(tc.tile_pool(name="dram", bufs=1, space="DRAM"))
        uhp = ctx.enter_context(tc.tile_pool(name="uh", bufs=1))
        tiny = ctx.enter_context(tc.tile_pool(name="ty", bufs=3))
        vp = ctx.enter_context(tc.tile_pool(name="vp", bufs=1))
        p1ctx = ExitStack()
        wpool = p1ctx.enter_context(tc.tile_pool(name="wp", bufs=1))
        xpool = p1ctx.enter_context(tc.tile_pool(name="xp", bufs=2))
        xtp = p1ctx.enter_context(tc.tile_pool(name="xtp", bufs=1))
        epsum = p1ctx.enter_context(tc.tile_pool(name="ep", bufs=3, space="PSUM"))
        s0psum = p1ctx.enter_context(tc.tile_pool(name="s0p", bufs=1, space="PSUM"))
        tmpp = p1ctx.enter_context(tc.tile_pool(name="tm", bufs=4))

        # bounce layout: addr = ((ch*16+bs)*8+il)*32768 + oc*2048 + sub*512 + e
        # write partitions (bs,il) stride 32768; read partitions (il,oc)
        # stride 2048 -> both sides are 3-dim APs.
        ub = dram.tile([NCH, BS, 8, NOCT, NSUB, JK], BF16)
        sstage = [dram.tile([2 * NG8, 128, JK], BF16, name=f"sst{r}")
                  for r in (1, 2)]
        arin = [dram.tile([B, JK], BF16, name=f"arin{r}") for r in range(3)]
        arout = [dram.tile([B, JK], BF16, name=f"arout{r}") for r in range(2)]
        rsout = dram.tile([B // NCORES, JK], BF16)
        wd = [dram.tile([B, JK], BF16, name=f"wd{r}") for r in (1, 2)]
        ar1h = [dram.tile([B // 2, JK], BF16, name=f"ar1h{h}") for h in (0, 1)]
        ao1h = [dram.tile([B // 2, JK], BF16, name=f"ao1h{h}") for h in (0, 1)]
        wd2h = [dram.tile([B // 2, JK], BF16, name=f"wd2h{h}") for h in (0, 1)]

        # resident tiles
        wres = wpool.tile([128, NCH, NOCT, JK], BF16, tag="wres")
        nc.sync.dma_start(wres[:, 0, 0:8], wl2_d[:, 0, 0:8])
        xT = xtp.tile([128, NCH, NOCT, B], BF16, tag="xT")

        def preload(ch, ocg):
            # stream the rest of W/xT behind the first compute-critical loads
            if ch == 0 and ocg == 1:
                nc.sync.dma_start(wres[:, 0, 8:16], wl2_d[:, 0, 8:16])
            elif ch == 0 and ocg == 2:
                nc.sync.dma_start(wres[:, 1, 0:8], wl2_d[:, 1, 0:8])
                nc.sync.dma_start(wres[:, 1, 8:16], wl2_d[:, 1, 8:16])
            elif ch == 0 and ocg == 3:
                nc.sync.dma_start(xT[:], xT2_d[:])
        u_hat = [uhp.tile([128, NSUB * BS, JK], BF16, tag=f"uh{c}",
                          name=f"u_hat{c}") for c in range(NCH)]

        # ---------------- P1: einsum + bounce; s0 between chunk passes ----
        # NOTE: GPSIMD/Pool cannot access PSUM on real HW -- drains must
        # stay on DVE/Act.
        drain_ch = [[nc.scalar, nc.vector], [nc.scalar, nc.vector]]
        s0ps = s0psum.tile([B, JK], F32, tag="s0ps")

        def einsum_ch(ch):
            for ocg in range(4):
                xbd = xpool.tile([128, 4, NSUB, 128], BF16, tag="xbd")
                nc.sync.dma_start(xbd[:], xbd2_d[:, ch, ocg * 4:(ocg + 1) * 4])
                preload(ch, ocg)
                for oo in range(4):
                    oc = ocg * 4 + oo
                    wt = wres[:, ch, oc, :]
                    tm4 = tmpp.tile([128, NSUB, JK], BF16, tag="tm4")
                    for half in range(2):
                        pe = epsum.tile([128, 2 * JK], F32, tag="pe")
                        for s2 in range(2):
                            sub = half * 2 + s2
                            nc.tensor.matmul(pe[:, s2 * JK:(s2 + 1) * JK],
                                             xbd[:, oo, sub, :], wt,
                                             start=True, stop=True,
                                             skip_group_check=True)
                        eng = drain_ch[ch][half]
                        dst = tm4[:, half * 2:(half + 1) * 2, :] \
                            .rearrange("p s f -> p (s f)")
                        if eng is nc.scalar:
                            eng.copy(dst, pe[:])
                        else:
                            eng.tensor_copy(dst, pe[:])
                    # bounce write: partitions (bs,il) at stride 32768
                    base = ub[:]
                    wdst = bass.AP(
                        tensor=base.tensor,
                        offset=(base.offset + ch * BS * 8 * 32768
                                + oc * NSUB * JK),
                        ap=[[NOCT * NSUB * JK, 128], [JK, NSUB], [1, JK]])
                    nc.sync.dma_start(wdst, tm4[:])

        def read_ch(ch, eng=None):
            # bounce read: partitions (il,oc) at stride 2048, per sub;
            # ch0 from the Act queue so SP can keep streaming ch1 writes,
            # ch1 from SP (runs right after its own writes)
            eng = eng or nc.scalar
            base = ub[:]
            for sub in range(NSUB):
                off = base.offset + ch * BS * 8 * 32768 + sub * JK
                src = bass.AP(tensor=base.tensor, offset=off,
                              ap=[[NSUB * JK, 128], [8 * 32768, BS], [1, JK]])
                eng.dma_start(
                    u_hat[ch][:, sub * BS:(sub + 1) * BS, :], src)

        einsum_ch(0)
        # s0 on PE after the ch0 einsum stream
        for ch in range(NCH):
            for oc in range(NOCT):
                nc.tensor.matmul(s0ps[:], xT[:, ch, oc, :], wres[:, ch, oc, :],
                                 start=(ch == 0 and oc == 0),
                                 stop=(ch == NCH - 1 and oc == NOCT - 1))
        read_ch(0)
        s0sb = vp.tile([B, JK], BF16, tag="s0sb")
        nc.scalar.mul(s0sb[:], s0ps[:], 1.0 / J)
        nc.scalar.dma_start(arin[0][:], s0sb[:])
        einsum_ch(1)

        def all_reduce(idx, last=False):
            if collectives:
                if last:
                    nc.gpsimd.collective_compute(
                        "ReduceScatter", OP.add,
                        replica_groups=[list(range(NCORES))],
                        ins=[arin[idx].opt()], outs=[rsout.opt()])
                else:
                    nc.gpsimd.collective_compute(
                        "AllReduce", OP.add,
                        replica_groups=[list(range(NCORES))],
                        ins=[arin[idx].opt()], outs=[arout[idx].opt()])
            else:
                if last:
                    nc.scalar.dma_start(rsout[:], arin[idx][:][0:B // NCORES])
                else:
                    nc.scalar.dma_start(arout[idx][:], arin[idx][:])

        all_reduce(0)

        # ---------------- rounds 1, 2 ----------------
        wacc = vp.tile([B, JK], F32, tag="wacc")

        def vchain(r):
            # squash previous round's s -> v_{r-1}; build w_r staging
            sv = vp.tile([B, JK], BF16, tag="sv")
            nc.scalar.dma_start(sv[:], arout[r - 1][:])
            v = _squash_emit(nc, vp, tiny, sv[:], BF16, tag=f"r{r}")
            if r == 1:
                nc.vector.tensor_copy(wacc[:], v[:])
                nc.scalar.dma_start(wd[0][:], v[:])
            else:
                nc.gpsimd.tensor_add(wacc[:], wacc[:], v[:])
                wb = vp.tile([B, JK], BF16, tag="wb")
                nc.vector.tensor_copy(wb[:], wacc[:])
                nc.scalar.dma_start(wd[1][:], wb[:])

        def all_reduce_h(h):
            if collectives:
                nc.gpsimd.collective_compute(
                    "AllReduce", OP.add,
                    replica_groups=[list(range(NCORES))],
                    ins=[ar1h[h].opt()], outs=[ao1h[h].opt()])
            else:
                nc.scalar.dma_start(ao1h[h][:], ar1h[h][:])

        def vchain2_half(h):
            # half-batch w2 staging: overlaps round 1's second half.
            # All tiles sit at base partition h*32 so TT inputs (incl. the
            # wacc slice) share equal base partitions (HW constraint).
            nb = B // 2
            off = h * nb
            sv = vp.tile([off + nb, JK], BF16, tag="sv2h")
            nc.scalar.dma_start(sv[off:off + nb, :], ao1h[h][:])
            v = _squash_emit(nc, vp, tiny, sv[off:off + nb, :], BF16,
                             nb=nb, tag=f"h{h}", off=off)
            wb = vp.tile([off + nb, JK], BF16, tag="wb2h")
            nc.vector.tensor_add(wb[off:off + nb, :],
                                 wacc[off:off + nb, :], v)
            nc.scalar.dma_start(wd2h[h][:], wb[off:off + nb, :])

        def round_body(r, mid_hooks=None):
            for g in range(NG8):
                if r == 1:
                    wsrc, woff = wd[0][:], g * 8 * JK
                else:
                    wsrc, woff = wd2h[g // 4][:], (g % 4) * 8 * JK
                # broadcast w[8b, 512] to all partitions
                wexp = wxp.tile([128, 8 * JK], BF16, tag="wexp")
                (nc.scalar if g == 0 else nc.gpsimd).dma_start(
                    wexp[:],
                    bass.AP(tensor=wsrc.tensor,
                            offset=wsrc.offset + woff,
                            ap=[[0, 128], [1, 8 * JK]]))
                pss = [spsum.tile([128, JK], F32, tag=f"ps{bq}",
                                  name=f"ps{r}_{g}_{bq}") for bq in range(2)]
                for ch in range(NCH):
                    uslc = u_hat[ch][:, g * 8:(g + 1) * 8, :] \
                        .rearrange("p b f -> p (b f)")
                    veng = nc.vector
                    prod = prp.tile([128, 8 * JK], BF16, tag="prod")
                    veng.tensor_mul(prod[:], uslc, wexp[:])
                    # k-reduction as a 2x-mode TT add tree (TensorReduce has
                    # no DVE perf modes -> 4x slower than this)
                    pv = prod[:].rearrange("p (bj k) -> p bj k", k=Kd)
                    t1 = smp.tile([128, 256, 8], BF16, tag="tr1")
                    veng.tensor_add(t1[:], pv[:, :, 0:8], pv[:, :, 8:16])
                    t2 = smp.tile([128, 256, 4], BF16, tag="tr2")
                    veng.tensor_add(t2[:], t1[:, :, 0:4], t1[:, :, 4:8])
                    t3 = smp.tile([128, 256, 2], BF16, tag="tr3")
                    veng.tensor_add(t3[:], t2[:, :, 0:2], t2[:, :, 2:4])
                    lg = smp.tile([128, 8, J], BF16, tag="lg")
                    veng.tensor_add(
                        lg[:].rearrange("p b j -> p (b j)"),
                        t3[:, :, 0], t3[:, :, 1])
                    # softmax, staged j-major so 1/z multiply stays 4x
                    est = smp.tile([128, J, 8], BF16, tag="est")
                    nc.scalar.activation(est[:].rearrange("p j b -> p b j"),
                                         lg[:], ACTF.Exp)
                    z = tiny.tile([128, 8], BF16, tag="z")
                    with nc.allow_low_precision("bf16 softmax z"):
                        nc.vector.tensor_reduce(
                            z[:], est[:].rearrange("p j b -> p b j"),
                            axis=AX.X, op=OP.add)
                    rz = tiny.tile([128, 8], BF16, tag="rz")
                    with nc.allow_low_precision("bf16 softmax 1/z"):
                        nc.vector.reciprocal(rz[:], z[:])
                    cst = smp.tile([128, J, 8], BF16, tag="cst")
                    nc.vector.tensor_mul(
                        cst[:], est[:],
                        rz[:, None, :].broadcast_to([128, J, 8]))
                    # c-weighted sums: 2 bq-groups x 4 col-tiled matmuls
                    for bq in range(2):
                        ps = pss[bq]
                        for b4 in range(4):
                            b = g * 8 + bq * 4 + b4
                            nc.tensor.matmul(
                                ps[b4 * 32:(b4 + 1) * 32, :],
                                cst[:, :, bq * 4 + b4],
                                u_hat[ch][:, b, :],
                                start=(ch == 0), stop=(ch == 1),
                                tile_position=(0, b4 * 32),
                                skip_group_check=True)
                        if ch == 1:
                            g2 = g * 2 + bq
                            sds = sdp.tile([128, JK], BF16, tag="sds")
                            nc.scalar.copy(sds[:], ps[:])
                            # pipelined staging + diag gather for this slice
                            nc.sync.dma_start(sstage[r - 1][g2], sds[:])
                            sb = sstage[r - 1][:]
                            diag = bass.AP(
                                tensor=sb.tensor,
                                offset=sb.offset + g2 * 128 * JK,
                                ap=[[32 * JK, 4], [JK + Kd, J], [1, Kd]])
                            if r == 1:
                                ab = ar1h[g // 4][:]
                                aoff = ((g % 4) * 8 + bq * 4) * JK
                            else:
                                ab = arin[r][:]
                                aoff = (g * 8 + bq * 4) * JK
                            adst = bass.AP(
                                tensor=ab.tensor, offset=ab.offset + aoff,
                                ap=[[JK, 4], [Kd, J], [1, Kd]])
                            nc.sync.dma_start(adst, diag)
                if mid_hooks and g in mid_hooks:
                    mid_hooks[g]()

        vchain(1)
        read_ch(1, nc.sync)
        p1ctx.close()
        spsum = ctx.enter_context(tc.tile_pool(name="sp", bufs=3, space="PSUM"))
        wxp = ctx.enter_context(tc.tile_pool(name="wx", bufs=3))
        prp = ctx.enter_context(tc.tile_pool(name="pr", bufs=2))
        smp = ctx.enter_context(tc.tile_pool(name="smp", bufs=2))
        sdp = ctx.enter_context(tc.tile_pool(name="sdp", bufs=6))
        round_body(1, mid_hooks={3: lambda: all_reduce_h(0),
                                 5: lambda: vchain2_half(0),
                                 7: lambda: (all_reduce_h(1),
                                             vchain2_half(1))})
        round_body(2)
        all_reduce(2, last=True)

        # final squash on this core's 8 batch rows
        svf = vp.tile([B // NCORES, JK], BF16, tag="svf")
        nc.scalar.dma_start(svf[:], rsout[:])
        vout = _squash_emit(nc, vp, tiny, svf[:], F32, nb=B // NCORES,
                            tag="fin")
        nc.scalar.dma_start(out_d[:].rearrange("b j k -> b (j k)"), vout)

    nc.compile()
    return nc


_NC_CACHE = None
_RUN_CACHE = None


def kernel(inputs, W, routings=3):
    """Full inputs in, full [B, J, K] output out. Shards over I across the
    8 NeuronCores internally; first call compiles and caches the executable."""
    global _NC_CACHE, _RUN_CACHE
    import jax
    from jax.sharding import NamedSharding, PartitionSpec
    inputs = np.asarray(inputs, dtype=np.float32)
    W = np.asarray(W, dtype=np.float32)
    if _NC_CACHE is None:
        _NC_CACHE = build_program()
    nc = _NC_CACHE
    if _RUN_CACHE is None:
        _RUN_CACHE = _build_sharded(nc)
    fn, mesh, in_names, out_names, out_avals, zero_outs = _RUN_CACHE
    per_core = [_host_prep(inputs, W, core) for core in range(NCORES)]
    sh = NamedSharding(mesh, PartitionSpec("core"))
    concat_in = [jax.device_put(
        np.concatenate([per_core[c][n] for c in range(NCORES)], axis=0), sh)
        for n in in_names]
    zeros = [jax.device_put(
        np.zeros((NCORES * z.shape[0], *z.shape[1:]), z.dtype), sh)
        for z in zero_outs]
    out = fn(*concat_in, *zeros)
    jax.block_until_ready(out)
    oidx = out_names.index("out")
    return np.asarray(out[oidx]).reshape(B, J, Kd)


# ---------------- timing harness (test-only) ----------------
def _build_sharded(nc):
    """Replicate bass2jax.run_bass_via_pjrt's jit construction, returning
    (fn, mesh, in_names, out_names, out_avals, zero_outs)."""
    import jax
    from jax.sharding import Mesh, PartitionSpec
    from jax.experimental.shard_map import shard_map
    from concourse.bass2jax import (_bass_exec_p, install_neuronx_cc_hook,
                                    partition_id_tensor)
    install_neuronx_cc_hook()
    partition_name = (nc.partition_id_tensor.name
                      if nc.partition_id_tensor else None)
    in_names, out_names, out_avals, zero_outs = [], [], [], []
    for alloc in nc.m.functions[0].allocations:
        if not isinstance(alloc, mybir.MemoryLocationSet):
            continue
        name = alloc.memorylocations[0].name
        if alloc.kind == "ExternalInput":
            if name != partition_name:
                in_names.append(name)
        elif alloc.kind == "ExternalOutput":
            out_names.append(name)
            shape = tuple(alloc.tensor_shape)
            dtype = mybir.dt.np(alloc.dtype)
            out_avals.append(jax.core.ShapedArray(shape, dtype))
            zero_outs.append(np.zeros(shape, dtype))
    n_params = len(in_names)
    n_outs = len(out_avals)
    all_in = list(in_names) + list(out_names)
    if partition_name is not None:
        all_in.append(partition_name)
    donate = tuple(range(n_params, n_params + n_outs))

    def _body(*args):
        operands = list(args)
        if partition_name is not None:
            operands.append(partition_id_tensor())
        return tuple(_bass_exec_p.bind(
            *operands, out_avals=tuple(out_avals), in_names=tuple(all_in),
            out_names=tuple(out_names), lowering_input_output_aliases=(),
            sim_require_finite=True, sim_require_nnan=True, nc=nc))

    devices = jax.devices()[:NCORES]
    mesh = Mesh(np.array(devices), ("core",))
    in_specs = (PartitionSpec("core"),) * (n_params + n_outs)
    out_specs = (PartitionSpec("core"),) * n_outs
    fn = jax.jit(shard_map(_body, mesh=mesh, in_specs=in_specs,
                           out_specs=out_specs, check_rep=False),
                 donate_argnums=donate, keep_unused=True)
    return fn, mesh, in_names[:n_params], out_names, out_avals, zero_outs


def timed_run(inputs, W, iters=20):
    """Returns (best_ns, times_ns list, output)."""
    import time, jax
    from jax.sharding import NamedSharding, PartitionSpec
    global _NC_CACHE, _RUN_CACHE
    if _NC_CACHE is None:
        _NC_CACHE = build_program()
    nc = _NC_CACHE
    if _RUN_CACHE is None:
        _RUN_CACHE = _build_sharded(nc)
    fn, mesh, in_names, out_names, out_avals, zero_outs = _RUN_CACHE
    per_core = [_host_prep(inputs, W, core) for core in range(NCORES)]
    sh = NamedSharding(mesh, PartitionSpec("core"))
    concat_in = [jax.device_put(
        np.concatenate([per_core[c][n] for c in range(NCORES)], axis=0), sh)
        for n in in_names]
    def make_zeros():
        return [jax.device_put(
            np.zeros((NCORES * z.shape[0], *z.shape[1:]), z.dtype), sh)
            for z in zero_outs]
    zsets = [make_zeros() for _ in range(iters + 3)]
    out = None
    times = []
    for it in range(iters + 3):
        t0 = time.perf_counter_ns()
        res = fn(*concat_in, *zsets[it])
        jax.block_until_ready(res)
        dt = time.perf_counter_ns() - t0
        if it >= 3:
            times.append(dt)
        out = res
    oidx = out_names.index("out")
    out_np = np.asarray(out[oidx]).reshape(B, J, Kd)
    return min(times), times, out_np


# revision 71
# speedup vs baseline: 1.7248x; 1.0039x over previous
"""CapsuleLayer (dynamic routing) Trainium2 kernel — 8 NeuronCores.

Strategy: shard over input capsules I (2048 -> 256/core); W load 4 MB/core
(bf16). Routing state is per-(b, i, j) and core-local; the three routing
reductions s_r are per-core PE partials, AllReduce'd in bf16 (64 KB).

Per-core pipeline (ordered to keep the in-order engine/DMA queues clear):
  P1  u_hat einsum: block-diag(x) [128,128] @ W-octet [128,512] matmuls,
      two N=512 col-tiled matmuls per 2-bank psum tile; psum drained as
      [128,1024] bf16 copies (DVE+Act only -- GPSIMD cannot touch PSUM);
      DRAM bounce with 3-dim APs on both sides (write partitions (bs,il)
      stride 32768, read partitions (il,oc) stride 2048) -> resident
      u_hat[ch] [i-part, (b, jk)] bf16, 128 KiB/partition.
      s0 = (1/J) sum_i u_hat as 32 direct xT @ W matmuls emitted between
      the two einsum chunk passes; its AllReduce + squash(v0) + w-staging
      complete while the bounce streams.
  P2  rounds 1,2 (8 groups of 8 batch x 2 i-chunks):
      logits = reduce_k(u_hat * w_bcast): TT multiply in 2x bf16 mode +
      a 4-level TT add tree (TensorReduce has no DVE perf modes and would
      be 4x slower); softmax staged j-major so the 1/z multiply stays in
      packed mode; c-weighted sums as 4 col-tiled PE matmuls per 4-batch
      sharing one psum tile across both i-chunks; per-slice staging DMA +
      7ns/descriptor diagonal gather DRAM->DRAM, pipelined into the round;
      AllReduce (r1) / ReduceScatter (r2) -> squash -> [8, 512] out.
      DMA issue is spread across queues: SP = bounce + staging + wexp-free
      slots, Act = bounce reads (ch0) + v-chain, Pool/SWDGE = wexp
      broadcasts, so no queue head-of-line-blocks another phase.
"""
import numpy as np
import ml_dtypes
from contextlib import ExitStack

import concourse.bass as bass
import concourse.mybir as mybir
import concourse.tile as tile
from concourse import bacc
from concourse import bass_utils

B, I, D, J, Kd = 64, 2048, 16, 32, 16
NCORES = 8
IC = I // NCORES      # 256 input capsules per core
NCH = 2               # i-chunks of 128 per core
NOCT = 16             # octets of 8 i per chunk
NSUB = 4              # sub-batches of b
BS = B // NSUB        # 16
JK = J * Kd           # 512
NG8 = 8               # groups of 8 batch elements
EPS = 1e-7
BF16 = mybir.dt.bfloat16
F32 = mybir.dt.float32
AX = mybir.AxisListType
OP = mybir.AluOpType
ACTF = mybir.ActivationFunctionType


def _host_prep(inputs, W, core):
    """Per-core DMA-ready layouts (bf16)."""
    bf = ml_dtypes.bfloat16
    Wc = W[core * IC:(core + 1) * IC]  # [256, 32, 16, 16] = [i, j, d, k]
    # wl2[(il,d), ch, oc, (j,k)]
    wl2 = Wc.reshape(NCH, NOCT, 8, J, D, Kd).transpose(2, 4, 0, 1, 3, 5) \
            .reshape(128, NCH, NOCT, JK)
    xc = inputs[:, core * IC:(core + 1) * IC, :]  # [64, 256, 16] = [b, i, d]
    xr = xc.reshape(B, NCH, NOCT, 8, D)           # [b, ch, oc, il, d]
    # xbd2[(il,d), ch, oc, sub, bs*8+il] = x[sub*16+bs, i(ch,oc,il), d]
    xbd2 = np.zeros((128, NCH, NOCT, NSUB, 128), np.float32)
    for il in range(8):
        blk = xr[:, :, :, il, :].reshape(NSUB, BS, NCH, NOCT, D) \
                                .transpose(4, 2, 3, 0, 1)
        xbd2[il * 16:(il + 1) * 16, :, :, :, il::8] = blk
    # xT2[(il,d), ch, oc, b]
    xT2 = np.ascontiguousarray(xr.transpose(3, 4, 1, 2, 0)).reshape(
        128, NCH, NOCT, B)
    return {"wl2": np.ascontiguousarray(wl2).astype(bf),
            "xbd2": xbd2.astype(bf),
            "xT2": np.ascontiguousarray(xT2).astype(bf)}


def _squash_emit(nc, pool, tiny, src_ap, out_dtype, nb=B, tag="", off=0):
    """squash on s [nb, 512] viewed [nb, 32, 16]; returns v tile.
    off: base partition for every tile (HW: TT inputs need equal bases)."""
    an = off + nb
    s_ = slice(off, an)
    sq = pool.tile([an, JK], BF16, tag="sq")
    nc.vector.tensor_mul(sq[s_], src_ap, src_ap)
    nn = tiny.tile([an, J], F32, tag="nn")
    nc.vector.tensor_reduce(nn[s_], sq[s_].rearrange("b (j k) -> b j k", k=Kd),
                            axis=AX.X, op=OP.add)
    t1 = tiny.tile([an, J], F32, tag="t1")
    nc.vector.tensor_scalar_add(t1[s_], nn[s_], 1.0)
    t2 = tiny.tile([an, J], F32, tag="t2")
    nc.vector.tensor_scalar_add(t2[s_], nn[s_], EPS)
    st = tiny.tile([an, J], F32, tag="st")
    nc.scalar.sqrt(st[s_], t2[s_])
    den = tiny.tile([an, J], F32, tag="den")
    nc.vector.tensor_mul(den[s_], t1[s_], st[s_])
    rden = tiny.tile([an, J], F32, tag="rden")
    nc.vector.reciprocal(rden[s_], den[s_])
    sc = tiny.tile([an, J], F32, tag="sc")
    nc.vector.tensor_mul(sc[s_], nn[s_], rden[s_])
    v = pool.tile([an, JK], out_dtype,
                  tag="vout" + ("f" if out_dtype == F32 else ""))
    nc.vector.tensor_mul(
        v[s_].rearrange("b (j k) -> b j k", k=Kd),
        src_ap.rearrange("b (j k) -> b j k", k=Kd),
        sc[s_][:, :, None].broadcast_to([nb, J, Kd]))
    return v[s_]


def build_program(collectives=True):
    nc = bacc.Bacc("TRN2", target_bir_lowering=False, debug=False,
                   num_devices=NCORES if collectives else 1)
    wl2_d = nc.dram_tensor("wl2", [128, NCH, NOCT, JK], BF16,
                           kind="ExternalInput")
    xbd2_d = nc.dram_tensor("xbd2", [128, NCH, NOCT, NSUB, 128], BF16,
                            kind="ExternalInput")
    xT2_d = nc.dram_tensor("xT2", [128, NCH, NOCT, B], BF16,
                           kind="ExternalInput")
    out_d = nc.dram_tensor("out", [B // NCORES, J, Kd], F32,
                           kind="ExternalOutput")

    with tile.TileContext(nc) as tc, ExitStack() as ctx:
        dram = ctx.enter_context(tc.tile_pool(name="dram", bufs=1, space="DRAM"))
        uhp = ctx.enter_context(tc.tile_pool(name="uh", bufs=1))
        tiny = ctx.enter_context(tc.tile_pool(name="ty", bufs=3))
        vp = ctx.enter_context(tc.tile_pool(name="vp", bufs=1))
        p1ctx = ExitStack()
        wpool = p1ctx.enter_context(tc.tile_pool(name="wp", bufs=1))
        xpool = p1ctx.enter_context(tc.tile_pool(name="xp", bufs=2))
        xtp = p1ctx.enter_context(tc.tile_pool(name="xtp", bufs=1))
        epsum = p1ctx.enter_context(tc.tile_pool(name="ep", bufs=3, space="PSUM"))
        s0psum = p1ctx.enter_context(tc.tile_pool(name="s0p", bufs=1, space="PSUM"))
        tmpp = p1ctx.enter_context(tc.tile_pool(name="tm", bufs=4))

        # bounce layout: addr = ((ch*16+bs)*8+il)*32768 + oc*2048 + sub*512 + e
        # write partitions (bs,il) stride 32768; read partitions (il,oc)
        # stride 2048 -> both sides are 3-dim APs.
        ub = dram.tile([NCH, BS, 8, NOCT, NSUB, JK], BF16)
        sstage = [dram.tile([2 * NG8, 128, JK], BF16, name=f"sst{r}")
                  for r in (1, 2)]
        arin = [dram.tile([B, JK], BF16, name=f"arin{r}") for r in range(3)]
        arout = [dram.tile([B, JK], BF16, name=f"arout{r}") for r in range(2)]
        rsout = dram.tile([B // NCORES, JK], BF16)
        wd = [dram.tile([B, JK], BF16, name=f"wd{r}") for r in (1, 2)]
        ar1h = [dram.tile([B // 2, JK], BF16, name=f"ar1h{h}") for h in (0, 1)]
        ao1h = [dram.tile([B // 2, JK], BF16, name=f"ao1h{h}") for h in (0, 1)]
        wd2h = [dram.tile([B // 2, JK], BF16, name=f"wd2h{h}") for h in (0, 1)]

        # resident tiles
        wres = wpool.tile([128, NCH, NOCT, JK], BF16, tag="wres")
        nc.sync.dma_start(wres[:, 0, 0:8], wl2_d[:, 0, 0:8])
        xT = xtp.tile([128, NCH, NOCT, B], BF16, tag="xT")

        def preload(ch, ocg):
            # stream the rest of W/xT behind the first compute-critical loads
            if ch == 0 and ocg == 1:
                nc.sync.dma_start(wres[:, 0, 8:16], wl2_d[:, 0, 8:16])
            elif ch == 0 and ocg == 2:
                nc.sync.dma_start(wres[:, 1, 0:8], wl2_d[:, 1, 0:8])
                nc.sync.dma_start(wres[:, 1, 8:16], wl2_d[:, 1, 8:16])
            elif ch == 0 and ocg == 3:
                nc.sync.dma_start(xT[:], xT2_d[:])
        u_hat = [uhp.tile([128, NSUB * BS, JK], BF16, tag=f"uh{c}",
                          name=f"u_hat{c}") for c in range(NCH)]

        # ---------------- P1: einsum + bounce; s0 between chunk passes ----
        # NOTE: GPSIMD/Pool cannot access PSUM on real HW -- drains must
        # stay on DVE/Act.
        drain_ch = [[nc.scalar, nc.vector], [nc.scalar, nc.vector]]
        s0ps = s0psum.tile([B, JK], F32, tag="s0ps")

        def einsum_ch(ch):
            for ocg in range(4):
                xbd = xpool.tile([128, 4, NSUB, 128], BF16, tag="xbd")
                nc.sync.dma_start(xbd[:], xbd2_d[:, ch, ocg * 4:(ocg + 1) * 4])
                preload(ch, ocg)
                for oo in range(4):
                    oc = ocg * 4 + oo
                    wt = wres[:, ch, oc, :]
                    tm4 = tmpp.tile([128, NSUB, JK], BF16, tag="tm4")
                    for half in range(2):
                        pe = epsum.tile([128, 2 * JK], F32, tag="pe")
                        for s2 in range(2):
                            sub = half * 2 + s2
                            nc.tensor.matmul(pe[:, s2 * JK:(s2 + 1) * JK],
                                             xbd[:, oo, sub, :], wt,
                                             start=True, stop=True,
                                             skip_group_check=True)
                        eng = drain_ch[ch][half]
                        dst = tm4[:, half * 2:(half + 1) * 2, :] \
                            .rearrange("p s f -> p (s f)")
                        if eng is nc.scalar:
                            eng.copy(dst, pe[:])
                        else:
                            eng.tensor_copy(dst, pe[:])
                    # bounce write: partitions (bs,il) at stride 32768
                    base = ub[:]
                    wdst = bass.AP(
                        tensor=base.tensor,
                        offset=(base.offset + ch * BS * 8 * 32768
                                + oc * NSUB * JK),
                        ap=[[NOCT * NSUB * JK, 128], [JK, NSUB], [1, JK]])
                    nc.sync.dma_start(wdst, tm4[:])

        def read_ch(ch, eng=None):
            # bounce read: partitions (il,oc) at stride 2048, per sub;
            # ch0 from the Act queue so SP can keep streaming ch1 writes,
            # ch1 from SP (runs right after its own writes)
            eng = eng or nc.scalar
            base = ub[:]
            for sub in range(NSUB):
                off = base.offset + ch * BS * 8 * 32768 + sub * JK
                src = bass.AP(tensor=base.tensor, offset=off,
                              ap=[[NSUB * JK, 128], [8 * 32768, BS], [1, JK]])
                eng.dma_start(
                    u_hat[ch][:, sub * BS:(sub + 1) * BS, :], src)

        einsum_ch(0)
        # s0 on PE after the ch0 einsum stream
        for ch in range(NCH):
            for oc in range(NOCT):
                nc.tensor.matmul(s0ps[:], xT[:, ch, oc, :], wres[:, ch, oc, :],
                                 start=(ch == 0 and oc == 0),
                                 stop=(ch == NCH - 1 and oc == NOCT - 1))
        read_ch(0)
        s0sb = vp.tile([B, JK], BF16, tag="s0sb")
        nc.scalar.mul(s0sb[:], s0ps[:], 1.0 / J)
        nc.scalar.dma_start(arin[0][:], s0sb[:])
        einsum_ch(1)

        def all_reduce(idx, last=False):
            if collectives:
                if last:
                    nc.gpsimd.collective_compute(
                        "ReduceScatter", OP.add,
                        replica_groups=[list(range(NCORES))],
                        ins=[arin[idx].opt()], outs=[rsout.opt()])
                else:
                    nc.gpsimd.collective_compute(
                        "AllReduce", OP.add,
                        replica_groups=[list(range(NCORES))],
                        ins=[arin[idx].opt()], outs=[arout[idx].opt()])
            else:
                if last:
                    nc.scalar.dma_start(rsout[:], arin[idx][:][0:B // NCORES])
                else:
                    nc.scalar.dma_start(arout[idx][:], arin[idx][:])

        all_reduce(0)

        # ---------------- rounds 1, 2 ----------------
        wacc = vp.tile([B, JK], F32, tag="wacc")

        def vchain(r):
            # squash previous round's s -> v_{r-1}; build w_r staging
            sv = vp.tile([B, JK], BF16, tag="sv")
            nc.scalar.dma_start(sv[:], arout[r - 1][:])
            v = _squash_emit(nc, vp, tiny, sv[:], BF16, tag=f"r{r}")
            if r == 1:
                nc.vector.tensor_copy(wacc[:], v[:])
                nc.scalar.dma_start(wd[0][:], v[:])
            else:
                nc.gpsimd.tensor_add(wacc[:], wacc[:], v[:])
                wb = vp.tile([B, JK], BF16, tag="wb")
                nc.vector.tensor_copy(wb[:], wacc[:])
                nc.scalar.dma_start(wd[1][:], wb[:])

        def all_reduce_h(h):
            if collectives:
                nc.gpsimd.collective_compute(
                    "AllReduce", OP.add,
                    replica_groups=[list(range(NCORES))],
                    ins=[ar1h[h].opt()], outs=[ao1h[h].opt()])
            else:
                nc.scalar.dma_start(ao1h[h][:], ar1h[h][:])

        def vchain2_half(h):
            # half-batch w2 staging: overlaps round 1's second half.
            # All tiles sit at base partition h*32 so TT inputs (incl. the
            # wacc slice) share equal base partitions (HW constraint).
            nb = B // 2
            off = h * nb
            sv = vp.tile([off + nb, JK], BF16, tag="sv2h")
            nc.scalar.dma_start(sv[off:off + nb, :], ao1h[h][:])
            v = _squash_emit(nc, vp, tiny, sv[off:off + nb, :], BF16,
                             nb=nb, tag=f"h{h}", off=off)
            wb = vp.tile([off + nb, JK], BF16, tag="wb2h")
            nc.vector.tensor_add(wb[off:off + nb, :],
                                 wacc[off:off + nb, :], v)
            nc.scalar.dma_start(wd2h[h][:], wb[off:off + nb, :])

        def round_body(r, mid_hooks=None):
            for g in range(NG8):
                if r == 1:
                    wsrc, woff = wd[0][:], g * 8 * JK
                else:
                    wsrc, woff = wd2h[g // 4][:], (g % 4) * 8 * JK
                # broadcast w[8b, 512] to all partitions
                wexp = wxp.tile([128, 8 * JK], BF16, tag="wexp")
                (nc.scalar if g == 0 else nc.gpsimd).dma_start(
                    wexp[:],
                    bass.AP(tensor=wsrc.tensor,
                            offset=wsrc.offset + woff,
                            ap=[[0, 128], [1, 8 * JK]]))
                pss = [spsum.tile([128, JK], F32, tag=f"ps{bq}",
                                  name=f"ps{r}_{g}_{bq}") for bq in range(2)]
                for ch in range(NCH):
                    uslc = u_hat[ch][:, g * 8:(g + 1) * 8, :] \
                        .rearrange("p b f -> p (b f)")
                    veng = nc.vector
                    prod = prp.tile([128, 8 * JK], BF16, tag="prod")
                    veng.tensor_mul(prod[:], uslc, wexp[:])
                    # k-reduction as a 2x-mode TT add tree (TensorReduce has
                    # no DVE perf modes -> 4x slower than this)
                    pv = prod[:].rearrange("p (bj k) -> p bj k", k=Kd)
                    t1 = smp.tile([128, 256, 8], BF16, tag="tr1")
                    veng.tensor_add(t1[:], pv[:, :, 0:8], pv[:, :, 8:16])
                    t2 = smp.tile([128, 256, 4], BF16, tag="tr2")
                    veng.tensor_add(t2[:], t1[:, :, 0:4], t1[:, :, 4:8])
                    t3 = smp.tile([128, 256, 2], BF16, tag="tr3")
                    veng.tensor_add(t3[:], t2[:, :, 0:2], t2[:, :, 2:4])
                    lg = smp.tile([128, 8, J], BF16, tag="lg")
                    veng.tensor_add(
                        lg[:].rearrange("p b j -> p (b j)"),
                        t3[:, :, 0], t3[:, :, 1])
                    # softmax, staged j-major so 1/z multiply stays 4x
                    est = smp.tile([128, J, 8], BF16, tag="est")
                    nc.scalar.activation(est[:].rearrange("p j b -> p b j"),
                                         lg[:], ACTF.Exp)
                    z = tiny.tile([128, 8], BF16, tag="z")
                    with nc.allow_low_precision("bf16 softmax z"):
                        nc.vector.tensor_reduce(
                            z[:], est[:].rearrange("p j b -> p b j"),
                            axis=AX.X, op=OP.add)
                    rz = tiny.tile([128, 8], BF16, tag="rz")
                    with nc.allow_low_precision("bf16 softmax 1/z"):
                        nc.vector.reciprocal(rz[:], z[:])
                    cst = smp.tile([128, J, 8], BF16, tag="cst")
                    nc.vector.tensor_mul(
                        cst[:], est[:],
                        rz[:, None, :].broadcast_to([128, J, 8]))
                    # c-weighted sums: 2 bq-groups x 4 col-tiled matmuls
                    for bq in range(2):
                        ps = pss[bq]
                        for b4 in range(4):
                            b = g * 8 + bq * 4 + b4
                            nc.tensor.matmul(
                                ps[b4 * 32:(b4 + 1) * 32, :],
                                cst[:, :, bq * 4 + b4],
                                u_hat[ch][:, b, :],
                                start=(ch == 0), stop=(ch == 1),
                                tile_position=(0, b4 * 32),
                                skip_group_check=True)
                        if ch == 1:
                            g2 = g * 2 + bq
                            sds = sdp.tile([128, JK], BF16, tag="sds")
                            nc.scalar.copy(sds[:], ps[:])
                            # pipelined staging + diag gather for this slice
                            nc.sync.dma_start(sstage[r - 1][g2], sds[:])
                            sb = sstage[r - 1][:]
                            diag = bass.AP(
                                tensor=sb.tensor,
                                offset=sb.offset + g2 * 128 * JK,
                                ap=[[32 * JK, 4], [JK + Kd, J], [1, Kd]])
                            if r == 1:
                                ab = ar1h[g // 4][:]
                                aoff = ((g % 4) * 8 + bq * 4) * JK
                            else:
                                ab = arin[r][:]
                                aoff = (g * 8 + bq * 4) * JK
                            adst = bass.AP(
                                tensor=ab.tensor, offset=ab.offset + aoff,
                                ap=[[JK, 4], [Kd, J], [1, Kd]])
                            nc.sync.dma_start(adst, diag)
                if mid_hooks and g in mid_hooks:
                    mid_hooks[g]()

        vchain(1)
        read_ch(1, nc.sync)
        p1ctx.close()
        spsum = ctx.enter_context(tc.tile_pool(name="sp", bufs=3, space="PSUM"))
        wxp = ctx.enter_context(tc.tile_pool(name="wx", bufs=3))
        prp = ctx.enter_context(tc.tile_pool(name="pr", bufs=2))
        smp = ctx.enter_context(tc.tile_pool(name="smp", bufs=2))
        sdp = ctx.enter_context(tc.tile_pool(name="sdp", bufs=6))
        round_body(1, mid_hooks={3: lambda: all_reduce_h(0),
                                 4: lambda: vchain2_half(0),
                                 7: lambda: (all_reduce_h(1),
                                             vchain2_half(1))})
        round_body(2)
        all_reduce(2, last=True)

        # final squash on this core's 8 batch rows
        svf = vp.tile([B // NCORES, JK], BF16, tag="svf")
        nc.scalar.dma_start(svf[:], rsout[:])
        vout = _squash_emit(nc, vp, tiny, svf[:], F32, nb=B // NCORES,
                            tag="fin")
        nc.scalar.dma_start(out_d[:].rearrange("b j k -> b (j k)"), vout)

    nc.compile()
    return nc


_NC_CACHE = None
_RUN_CACHE = None


def kernel(inputs, W, routings=3):
    """Full inputs in, full [B, J, K] output out. Shards over I across the
    8 NeuronCores internally; first call compiles and caches the executable."""
    global _NC_CACHE, _RUN_CACHE
    import jax
    from jax.sharding import NamedSharding, PartitionSpec
    inputs = np.asarray(inputs, dtype=np.float32)
    W = np.asarray(W, dtype=np.float32)
    if _NC_CACHE is None:
        _NC_CACHE = build_program()
    nc = _NC_CACHE
    if _RUN_CACHE is None:
        _RUN_CACHE = _build_sharded(nc)
    fn, mesh, in_names, out_names, out_avals, zero_outs = _RUN_CACHE
    per_core = [_host_prep(inputs, W, core) for core in range(NCORES)]
    sh = NamedSharding(mesh, PartitionSpec("core"))
    concat_in = [jax.device_put(
        np.concatenate([per_core[c][n] for c in range(NCORES)], axis=0), sh)
        for n in in_names]
    zeros = [jax.device_put(
        np.zeros((NCORES * z.shape[0], *z.shape[1:]), z.dtype), sh)
        for z in zero_outs]
    out = fn(*concat_in, *zeros)
    jax.block_until_ready(out)
    oidx = out_names.index("out")
    return np.asarray(out[oidx]).reshape(B, J, Kd)


# ---------------- timing harness (test-only) ----------------
def _build_sharded(nc):
    """Replicate bass2jax.run_bass_via_pjrt's jit construction, returning
    (fn, mesh, in_names, out_names, out_avals, zero_outs)."""
    import jax
    from jax.sharding import Mesh, PartitionSpec
    from jax.experimental.shard_map import shard_map
    from concourse.bass2jax import (_bass_exec_p, install_neuronx_cc_hook,
                                    partition_id_tensor)
    install_neuronx_cc_hook()
    partition_name = (nc.partition_id_tensor.name
                      if nc.partition_id_tensor else None)
    in_names, out_names, out_avals, zero_outs = [], [], [], []
    for alloc in nc.m.functions[0].allocations:
        if not isinstance(alloc, mybir.MemoryLocationSet):
            continue
        name = alloc.memorylocations[0].name
        if alloc.kind == "ExternalInput":
            if name != partition_name:
                in_names.append(name)
        elif alloc.kind == "ExternalOutput":
            out_names.append(name)
            shape = tuple(alloc.tensor_shape)
            dtype = mybir.dt.np(alloc.dtype)
            out_avals.append(jax.core.ShapedArray(shape, dtype))
            zero_outs.append(np.zeros(shape, dtype))
    n_params = len(in_names)
    n_outs = len(out_avals)
    all_in = list(in_names) + list(out_names)
    if partition_name is not None:
        all_in.append(partition_name)
    donate = tuple(range(n_params, n_params + n_outs))

    def _body(*args):
        operands = list(args)
        if partition_name is not None:
            operands.append(partition_id_tensor())
        return tuple(_bass_exec_p.bind(
            *operands, out_avals=tuple(out_avals), in_names=tuple(all_in),
            out_names=tuple(out_names), lowering_input_output_aliases=(),
            sim_require_finite=True, sim_require_nnan=True, nc=nc))

    devices = jax.devices()[:NCORES]
    mesh = Mesh(np.array(devices), ("core",))
    in_specs = (PartitionSpec("core"),) * (n_params + n_outs)
    out_specs = (PartitionSpec("core"),) * n_outs
    fn = jax.jit(shard_map(_body, mesh=mesh, in_specs=in_specs,
                           out_specs=out_specs, check_rep=False),
                 donate_argnums=donate, keep_unused=True)
    return fn, mesh, in_names[:n_params], out_names, out_avals, zero_outs


def timed_run(inputs, W, iters=20):
    """Returns (best_ns, times_ns list, output)."""
    import time, jax
    from jax.sharding import NamedSharding, PartitionSpec
    global _NC_CACHE, _RUN_CACHE
    if _NC_CACHE is None:
        _NC_CACHE = build_program()
    nc = _NC_CACHE
    if _RUN_CACHE is None:
        _RUN_CACHE = _build_sharded(nc)
    fn, mesh, in_names, out_names, out_avals, zero_outs = _RUN_CACHE
    per_core = [_host_prep(inputs, W, core) for core in range(NCORES)]
    sh = NamedSharding(mesh, PartitionSpec("core"))
    concat_in = [jax.device_put(
        np.concatenate([per_core[c][n] for c in range(NCORES)], axis=0), sh)
        for n in in_names]
    def make_zeros():
        return [jax.device_put(
            np.zeros((NCORES * z.shape[0], *z.shape[1:]), z.dtype), sh)
            for z in zero_outs]
    zsets = [make_zeros() for _ in range(iters + 3)]
    out = None
    times = []
    for it in range(iters + 3):
        t0 = time.perf_counter_ns()
        res = fn(*concat_in, *zsets[it])
        jax.block_until_ready(res)
        dt = time.perf_counter_ns() - t0
        if it >= 3:
            times.append(dt)
        out = res
    oidx = out_names.index("out")
    out_np = np.asarray(out[oidx]).reshape(B, J, Kd)
    return min(times), times, out_np
